# revision 1
# baseline (speedup 1.0000x reference)
"""Two-layer GCN (GCNConv x2 + log_softmax) on 8 Trainium2 NeuronCores.

Strategy (graph/data parallel, nodes sharded 8 ways):
  - Norm factors dinv[src]*dinv[dst] factor into a pre-scale of the gather
    table rows and a post-scale of the aggregated output, so aggregation is a
    pure unweighted segment-sum of gathered 256B rows.
  - Layer tables (T1 = (dinv*x)@W1, T3 = allgather((dinv*relu(out1))@W2)) are
    node-major [N, 128] bf16 in DRAM (values in the first 64 cols, upper half
    never read -- dma_gather needs 256B rows); per-edge rows are fetched with
    gpsimd.dma_gather (int16 indices -> 4 windows of 25k rows,
    single_packet=False: single-packet mode crashes above ~1024 rows/call).
  - Segment-sum via selection-matrix matmuls: for each column of 128 messages
    belonging to one 128-node dst group, S[m, r] = (localid[m] == r) built
    on-device with a broadcast is_equal against an iota row; TensorE
    accumulates norm'd messages into the group's PSUM accumulator.
  - Layer 1 runs feature-major (M as lhsT) so T2^T feeds the W2 matmul
    directly; layer 2 runs node-major (S as lhsT) so log_softmax reduces
    along the free axis.
  - One AllGather between the layers exchanges the [12500, 64] f32 slices.
"""

import math
import os
from contextlib import ExitStack
from dataclasses import dataclass

import numpy as np
import ml_dtypes

import concourse.bass as bass
import concourse.tile as tile
from concourse import bacc, mybir
from concourse.bass_utils import run_bass_kernel_spmd

F32 = mybir.dt.float32
BF16 = mybir.dt.bfloat16
I16 = mybir.dt.int16
AF = mybir.ActivationFunctionType
ALU = mybir.AluOpType


@dataclass
class Cfg:
    n: int = 100000        # nodes
    nin: int = 128         # input features
    hid: int = 64          # hidden features (= table row width, 256B f32)
    outf: int = 40         # output features
    ncores: int = 8
    nwin: int = 4          # gather-table windows (int16 idx range)
    g: int = 128           # dst group size
    chunk_g: int = 12      # groups per PSUM chunk
    sub: int = 32          # columns per S-build/cast sub-slab
    xchunk: int = 2048     # nodes per T1-build matmul chunk

    @property
    def per(self):
        return self.n // self.ncores

    @property
    def win(self):
        return self.n // self.nwin

    @property
    def ng(self):
        return math.ceil(self.per / self.g)

    @property
    def perp(self):
        return self.ng * self.g


# ---------------------------------------------------------------- host side


def _preprocess(x, edge_index, W1, b1, W2, b2, cfg: Cfg):
    n, per, g, win = cfg.n, cfg.per, cfg.g, cfg.win
    nc_, ng, nwin = cfg.ncores, cfg.ng, cfg.nwin

    loops = np.arange(n, dtype=np.int64)
    src = np.concatenate([edge_index[0].astype(np.int64), loops])
    dst = np.concatenate([edge_index[1].astype(np.int64), loops])

    deg = np.bincount(dst, minlength=n).astype(np.float64)
    dinv = np.where(deg > 0, 1.0 / np.sqrt(deg), 0.0).astype(np.float32)

    # table pre-scale folded in on host
    xs = (x * dinv[:, None]).astype(np.float32)
    xsT = np.ascontiguousarray(xs.T).astype(ml_dtypes.bfloat16)  # [nin, n]

    # ---- per-core edge buckets ----
    core = dst // per
    gidx = (dst % per) // g
    widx = src // win
    lid = (dst % per) % g

    # counts[c, g, w]
    counts = np.zeros((nc_, ng, nwin), dtype=np.int64)
    np.add.at(counts, (core, gidx, widx), 1)
    ncols_gw = np.ceil(counts / g).max(axis=0).astype(np.int64)  # [ng, nwin]

    # chunk layout: for K: for w: for g in K: ncols_gw[g, w] columns
    chunks = [
        list(range(k0, min(k0 + cfg.chunk_g, ng))) for k0 in range(0, ng, cfg.chunk_g)
    ]
    # stream metadata
    call_meta = []  # per (K, w): (slot_off, n_slots, [(gid, ncols), ...])
    region_off = {}  # (g, w) -> slot offset of its region
    off = 0
    for K in chunks:
        for w in range(nwin):
            groups = [(gg, int(ncols_gw[gg, w])) for gg in K if ncols_gw[gg, w] > 0]
            sl0 = off
        # register region offsets
            for gg, ncol in groups:
                region_off[(gg, w)] = off
                off += ncol * g
            call_meta.append((sl0, off - sl0, groups))
    tot_slots = off
    tot_cols = tot_slots // g

    # ---- per-core idx / localid arrays ----
    order = np.lexsort((src, widx, gidx, core))  # sort by core, g, w, src
    src_s, core_s = src[order], core[order]
    g_s, w_s, lid_s = gidx[order], widx[order], lid[order]

    idx_all = np.zeros((nc_, tot_slots), dtype=np.int16)
    lid_all = np.full((nc_, tot_slots), 255.0, dtype=np.float32)
    for c in range(nc_):
        m = core_s == c
        sc, gc, wc, lc = src_s[m], g_s[m], w_s[m], lid_s[m]
        # slot position: region_off[(g, w)] + rank within (g, w)
        # compute rank within each (g,w) run (data is sorted by (g,w))
        key = gc * nwin + wc
        # run-start indices
        change = np.r_[True, key[1:] != key[:-1]]
        run_id = np.cumsum(change) - 1
        run_start = np.flatnonzero(change)
        rank = np.arange(len(key)) - run_start[run_id]
        base = np.array([region_off[(gg, ww)] for gg, ww in zip(gc[change], wc[change])])
        slot = base[run_id] + rank
        idx_all[c, slot] = (sc - wc * win).astype(np.int16)
        lid_all[c, slot] = lc

    # wrap idx into [128, tot_slots//16] (16-partition wrap, replicated x8)
    idx_wrap = np.zeros((nc_, 128, tot_slots // 16), dtype=np.int16)
    lid_cols = np.zeros((nc_, 128, tot_cols), dtype=ml_dtypes.bfloat16)
    for c in range(nc_):
        wrapped = idx_all[c].reshape(-1, 16).T  # [16, S/16]
        idx_wrap[c] = np.tile(wrapped, (8, 1))
        lid_cols[c] = lid_all[c].reshape(tot_cols, g).T.astype(ml_dtypes.bfloat16)

    # ---- small constants ----
    iota = np.tile(np.arange(g, dtype=np.float32), (g, 1)).astype(ml_dtypes.bfloat16)
    W1bf = W1.astype(ml_dtypes.bfloat16)  # [nin, hid]
    W2p = np.zeros((cfg.hid, cfg.hid), dtype=np.float32)
    W2p[:, : cfg.outf] = W2
    W2bf = W2p.astype(ml_dtypes.bfloat16)
    b1col = b1.reshape(cfg.hid, 1).astype(np.float32)
    b2bc = np.zeros((128, cfg.hid), dtype=np.float32)
    b2bc[:, : cfg.outf] = b2[None, :]

    dinv_bc = np.zeros((nc_, cfg.hid, cfg.perp), dtype=np.float32)
    dinv_col = np.zeros((nc_, 128, ng), dtype=np.float32)
    for c in range(nc_):
        dslice = np.concatenate(
            [dinv[c * per : (c + 1) * per], np.ones(cfg.perp - per, np.float32)]
        )
        dinv_bc[c] = np.tile(dslice, (cfg.hid, 1))
        dinv_col[c] = dslice.reshape(ng, g).T

    in_maps = []
    for c in range(nc_):
        in_maps.append(
            {
                "xsT": np.asarray(xsT),
                "W1bf": np.asarray(W1bf),
                "W2bf": np.asarray(W2bf),
                "b1col": b1col,
                "b2bc": b2bc,
                "iota": np.asarray(iota),
                "idx": idx_wrap[c],
                "lid": np.asarray(lid_cols[c]),
                "dinv_bc": dinv_bc[c],
                "dinv_col": dinv_col[c],
            }
        )

    sched = dict(call_meta=call_meta, chunks=chunks, ncols_gw=ncols_gw,
                 tot_slots=tot_slots, tot_cols=tot_cols)
    return in_maps, sched


# ---------------------------------------------------------------- device side


def _build(cfg: Cfg, sched) -> bacc.Bacc:
    n, hid, g, nwin, win = cfg.n, cfg.hid, cfg.g, cfg.nwin, cfg.win
    ng, per, perp = cfg.ng, cfg.per, cfg.perp
    call_meta, chunks = sched["call_meta"], sched["chunks"]
    tot_slots, tot_cols = sched["tot_slots"], sched["tot_cols"]
    ncols_gw = sched["ncols_gw"]
    max_call_cols = max((m[1] // g for m in call_meta), default=1)

    nc = bacc.Bacc("TRN2", target_bir_lowering=False, debug=False,
                   num_devices=cfg.ncores)

    xsT = nc.dram_tensor("xsT", [cfg.nin, n], BF16, kind="ExternalInput").ap()
    W1bf = nc.dram_tensor("W1bf", [cfg.nin, hid], BF16, kind="ExternalInput").ap()
    W2bf = nc.dram_tensor("W2bf", [hid, hid], BF16, kind="ExternalInput").ap()
    b1col = nc.dram_tensor("b1col", [hid, 1], F32, kind="ExternalInput").ap()
    b2bc = nc.dram_tensor("b2bc", [128, hid], F32, kind="ExternalInput").ap()
    iota_d = nc.dram_tensor("iota", [g, g], BF16, kind="ExternalInput").ap()
    idx_d = nc.dram_tensor("idx", [128, tot_slots // 16], I16, kind="ExternalInput").ap()
    lid_d = nc.dram_tensor("lid", [128, tot_cols], BF16, kind="ExternalInput").ap()
    dinv_bc_d = nc.dram_tensor("dinv_bc", [hid, perp], F32, kind="ExternalInput").ap()
    dinv_col_d = nc.dram_tensor("dinv_col", [128, ng], F32, kind="ExternalInput").ap()

    out_d = nc.dram_tensor("out", [per, cfg.outf], F32, kind="ExternalOutput").ap()

    # Tables are [n, 2*hid] bf16: first hid cols hold values, upper half is
    # never read (dma_gather needs 256B rows; the pad halves are garbage).
    T1 = nc.dram_tensor("T1", [n, 2 * hid], BF16).ap()
    # Compact AllGather: ship only the outf real output columns, then expand
    # locally into the 256B-row gather table (pad columns are never read).
    of_ = cfg.outf
    h2c_b = nc.dram_tensor("h2c_b", [per, of_], BF16).ap()         # AG input bounce
    T3c = nc.dram_tensor("T3c", [n, of_], BF16, addr_space="Shared").ap()
    T3 = nc.dram_tensor("T3", [n, 2 * hid], BF16).ap()             # expanded table

    from concourse import library_config

    with tile.TileContext(nc) as tc, ExitStack() as ctx:
        nc.gpsimd.load_library(library_config.mlp)

        consts = ctx.enter_context(tc.tile_pool(name="consts", bufs=1))
        sb = ctx.enter_context(tc.tile_pool(name="sb", bufs=3))
        subp = ctx.enter_context(tc.tile_pool(name="subp", bufs=3))
        eptmp = ctx.enter_context(tc.tile_pool(name="eptmp", bufs=4))
        psum_bank = ctx.enter_context(tc.tile_pool(name="psumb", bufs=6, space="PSUM"))
        psum_mm = ctx.enter_context(tc.tile_pool(name="psummm", bufs=2, space="PSUM"))

        # resident constants
        w1_t = consts.tile([cfg.nin, hid], BF16)
        nc.sync.dma_start(w1_t[:], W1bf[:, :])
        w2_t = consts.tile([hid, hid], BF16)
        nc.sync.dma_start(w2_t[:], W2bf[:, :])
        b1_t = consts.tile([hid, 1], F32)
        nc.sync.dma_start(b1_t[:], b1col[:, :])
        b2_t = consts.tile([128, hid], F32)
        nc.sync.dma_start(b2_t[:], b2bc[:, :])
        iota_t = consts.tile([g, g], BF16)
        nc.sync.dma_start(iota_t[:], iota_d[:, :])
        # dense repeated iota [128, sub*g] so the S-build tensor_tensor has a
        # dense stride-1 first operand (DVE 2x eligibility)
        iota_rep = consts.tile([g, cfg.sub * g], BF16)
        for _s in range(cfg.sub):
            nc.vector.tensor_copy(iota_rep[:, _s * g : (_s + 1) * g], iota_t[:])
        dinvbc_t = consts.tile([hid, perp], F32)
        nc.sync.dma_start(dinvbc_t[:], dinv_bc_d[:, :])
        dinvcol_t = consts.tile([128, ng], F32)
        nc.sync.dma_start(dinvcol_t[:], dinv_col_d[:, :])
        t2t = consts.tile([hid, perp], BF16)  # T2^T staging (layer-1 output)
        # resident localid stream (shared by both layers); idx slices are
        # DMA'd per call (residency would cost 36KB/partition of SBUF)
        lid_t = consts.tile([128, tot_cols], BF16)
        nc.sync.dma_start(lid_t[:], lid_d[:, :])

        # -------------------------------------------------- T1 build
        # xchunk nodes per iteration: one input DMA, xchunk/128 matmuls into
        # psum banks of 8 x [128, 64], bank-wise ACT evictions, one fused 3D
        # output DMA (DRAM view [nsub, 128, hid] <- SBUF [128, nsub, hid]).
        xc = cfg.xchunk
        for ch0 in range(0, n, xc):
            cols = min(xc, n - ch0)
            nsub = math.ceil(cols / 128)
            xt = sb.tile([cfg.nin, xc], BF16, tag="xt")
            nc.sync.dma_start(xt[:, :cols], xsT[:, ch0 : ch0 + cols])
            st = sb.tile([128, xc // 128, hid], BF16, tag="t1s")
            for b0 in range(0, nsub, 8):
                bn = min(8, nsub - b0)
                pt = psum_mm.tile([128, 512], F32, tag="t1p", name="pt")
                for qi in range(bn):
                    q = b0 + qi
                    qc = min(128, cols - q * 128)
                    nc.tensor.matmul(
                        out=pt[:qc, qi * hid : qi * hid + hid],
                        lhsT=xt[:, q * 128 : q * 128 + qc],
                        rhs=w1_t[:],
                        start=True,
                        stop=True,
                    )
                nc.scalar.copy(
                    st[:, b0 : b0 + bn, :].rearrange("p q h -> p (q h)"),
                    pt[:, : bn * hid],
                )
            # fused transposed store into the value halves of T1 rows
            if cols % 128 == 0:
                dram_view = T1[ch0 : ch0 + cols, :hid].rearrange(
                    "(q p) h -> p q h", p=128
                )
                nc.sync.dma_start(dram_view, st[:, :nsub, :])
            else:
                for q in range(nsub):
                    qc = min(128, cols - q * 128)
                    nc.sync.dma_start(
                        T1[ch0 + q * 128 : ch0 + q * 128 + qc, :hid],
                        st[:qc, q, :],
                    )

        # -------------------------------------------------- aggregation layers
        def agg_layer(layer: int, table_ap):
            """layer 1: feature-major accum [hid, 128]; writes t2t + h2s.
            layer 2: node-major accum [128, hid]; writes log_softmax to out."""
            gper = 4 if layer == 1 else 8  # accumulator regions per PSUM bank
            ci = 0
            for K in chunks:
                # per-group accumulator sub-regions inside full-bank tiles
                nbank = math.ceil(len(K) / gper)
                banks = [
                    psum_bank.tile([128, 512], F32, tag="acc", name=f"acc{layer}")
                    for _ in range(nbank)
                ]

                def acc_ap(j):
                    b = banks[j // gper]
                    if layer == 1:
                        r = j % 4
                        return b[0:64, r * 128 : r * 128 + 128]
                    r = j % 8
                    return b[:, r * 64 : r * 64 + 64]

                # start/stop are BANK-granular: start=True clears has_written
                # for the whole bank, so only the first matmul into each bank
                # tile uses start=True and only the last uses stop=True;
                # per-region init relies on per-element overwrite semantics.
                tot_bank = [0] * nbank
                for j, gg in enumerate(K):
                    tot_bank[j // gper] += int(
                        sum(ncols_gw[gg, w] for w in range(nwin))
                    )
                emitted_bank = [0] * nbank
                for w in range(nwin):
                    sl0, nsl, groups = call_meta[ci]
                    ci += 1
                    if nsl == 0:
                        continue
                    cols = nsl // g
                    it = sb.tile([128, max_call_cols * 8], I16, tag="idx")
                    nc.sync.dma_start(
                        it[:, : nsl // 16], idx_d[:, sl0 // 16 : (sl0 + nsl) // 16]
                    )
                    mt = sb.tile([128, max_call_cols, 2 * hid], BF16, tag="m")
                    nc.gpsimd.dma_gather(
                        mt[:, :cols, :],
                        table_ap[w * win : (w + 1) * win, :],
                        it[:, : nsl // 16],
                        nsl,
                        nsl,
                        2 * hid,
                        single_packet=False,
                    )
                    # column -> (group-in-K index) map
                    colg = []
                    for gg, ncol in groups:
                        colg += [K.index(gg)] * ncol
                    for s0 in range(0, cols, cfg.sub):
                        sc = min(cfg.sub, cols - s0)
                        c0 = sl0 // g + s0
                        st_ = subp.tile([128, cfg.sub * g], BF16, tag="sel")
                        nc.vector.tensor_tensor(
                            out=st_[:, : sc * g].rearrange("p (c r) -> p c r", r=g),
                            in0=iota_rep[:, : sc * g].rearrange(
                                "p (c r) -> p c r", r=g
                            ),
                            in1=lid_t[:, c0 : c0 + sc].to_broadcast([128, sc, g]),
                            op=ALU.is_equal,
                        )
                        for j in range(sc):
                            gj = colg[s0 + j]
                            b = gj // gper
                            first = emitted_bank[b] == 0
                            emitted_bank[b] += 1
                            last = emitted_bank[b] == tot_bank[b]
                            if layer == 1:
                                nc.tensor.matmul(
                                    out=acc_ap(gj),
                                    lhsT=mt[:, s0 + j, :hid],
                                    rhs=st_[:, j * g : (j + 1) * g],
                                    start=first,
                                    stop=last,
                                )
                            else:
                                nc.tensor.matmul(
                                    out=acc_ap(gj),
                                    lhsT=st_[:, j * g : (j + 1) * g],
                                    rhs=mt[:, s0 + j, :hid],
                                    start=first,
                                    stop=last,
                                )
                # epilogues for chunk K
                if layer == 2:
                    ot_stage = sb.tile(
                        [128, cfg.chunk_g, cfg.outf], F32, tag="otst", name="ot_stage"
                    )
                for j, gg in enumerate(K):
                    rows = min(g, per - gg * g)  # real dst rows in group
                    if layer == 1:
                        dslice = dinvbc_t[:, gg * g : gg * g + g]
                        t1_ = eptmp.tile([hid, g], F32, tag="ep1")
                        nc.vector.tensor_mul(t1_[:], acc_ap(j)[:], dslice[:])
                        t2_ = eptmp.tile([hid, g], F32, tag="ep2")
                        nc.scalar.activation(t2_[:], t1_[:], AF.Relu, bias=b1_t[:, :1])
                        nc.vector.tensor_mul(
                            t2t[:, gg * g : gg * g + g], t2_[:], dslice[:]
                        )
                    else:
                        t1_ = eptmp.tile([128, hid], F32, tag="ep1")
                        nc.vector.tensor_scalar(
                            t1_[:], acc_ap(j)[:], dinvcol_t[:, gg : gg + 1], None,
                            ALU.mult,
                        )
                        t2_ = eptmp.tile([128, hid], F32, tag="ep2")
                        nc.vector.tensor_add(t2_[:], t1_[:], b2_t[:])
                        of = cfg.outf
                        nmax = eptmp.tile([128, 1], F32, tag="nmax")
                        nc.vector.tensor_reduce(
                            nmax[:], t2_[:, :of], mybir.AxisListType.X, ALU.max,
                            negate=True,
                        )
                        ex = eptmp.tile([128, of], F32, tag="ex")
                        nc.scalar.activation(ex[:], t2_[:, :of], AF.Exp, bias=nmax[:, :1])
                        sm = eptmp.tile([128, 1], F32, tag="sm")
                        nc.vector.tensor_reduce(
                            sm[:], ex[:], mybir.AxisListType.X, ALU.add
                        )
                        ls = eptmp.tile([128, 1], F32, tag="ls")
                        nc.scalar.activation(ls[:], sm[:], AF.Ln)
                        nc.vector.tensor_scalar(
                            ot_stage[:, j, :], t2_[:, :of], nmax[:, :1], ls[:, :1],
                            ALU.add, ALU.subtract,
                        )
                if layer == 2:
                    # one fused store for the chunk's full groups, small store
                    # for a trailing partial group
                    nfull = sum(1 for gg in K if per - gg * g >= g)
                    r0 = K[0] * g
                    if nfull:
                        nc.sync.dma_start(
                            out_d[r0 : r0 + nfull * g, :].rearrange(
                                "(q p) f -> p q f", p=128
                            ),
                            ot_stage[:, :nfull, :],
                        )
                    for j, gg in enumerate(K):
                        rows = per - gg * g
                        if rows < g:
                            nc.sync.dma_start(
                                out_d[gg * g : gg * g + rows, :],
                                ot_stage[:rows, j, :],
                            )

        _phases = int(os.environ.get("GCN_PHASES", "4"))  # 1=t1 2=+l1 3=+ag 4=all
        if _phases >= 2:
            agg_layer(1, T1)

        # -------------------------------------------------- W2 matmul + AllGather
        if _phases >= 3:
            for g0 in range(0, ng, 8):
                gn = min(8, ng - g0)
                pw = psum_mm.tile([128, 512], F32, tag="t1p", name="pw")
                for qi in range(gn):
                    gg = g0 + qi
                    nc.tensor.matmul(
                        out=pw[:, qi * hid : qi * hid + hid],
                        lhsT=t2t[:, gg * g : gg * g + g],
                        rhs=w2_t[:],
                        start=True,
                        stop=True,
                    )
                hw = sb.tile([128, 8, hid], BF16, tag="h2t")
                nc.scalar.copy(
                    hw[:, :gn, :].rearrange("p q h -> p (q h)"), pw[:, : gn * hid]
                )
                nfull = sum(1 for gg in range(g0, g0 + gn) if per - gg * g >= g)
                if nfull:
                    nc.sync.dma_start(
                        h2c_b[g0 * g : (g0 + nfull) * g, :].rearrange(
                            "(q p) h -> p q h", p=128
                        ),
                        hw[:, :nfull, :of_],
                    )
                for qi in range(gn):
                    gg = g0 + qi
                    rows = per - gg * g
                    if rows < g:
                        nc.sync.dma_start(
                            h2c_b[gg * g : gg * g + rows, :], hw[:rows, qi, :of_]
                        )

        if _phases >= 3 and not os.environ.get("GCN_NO_COLL"):
            nc.gpsimd.collective_compute(
                "AllGather",
                ALU.bypass,
                replica_groups=[list(range(cfg.ncores))],
                ins=[h2c_b.opt()],
                outs=[T3c.opt()],
            )
            # expand compact rows into the 256B-row gather table; the pad
            # columns of T3 stay uninitialized and are never read.
            for r0 in range(0, n, per):
                nc.sync.dma_start(T3[r0 : r0 + per, :of_], T3c[r0 : r0 + per, :])

        if _phases >= 4:
            agg_layer(2, T3)

    nc.compile()
    return nc


# ---------------------------------------------------------------- entry


def kernel(x, edge_index, W1, b1, W2, b2, cfg: Cfg | None = None, _run=None):
    cfg = cfg or Cfg()
    in_maps, sched = _preprocess(
        np.asarray(x), np.asarray(edge_index), np.asarray(W1), np.asarray(b1),
        np.asarray(W2), np.asarray(b2), cfg
    )
    nc = _build(cfg, sched)
    if _run is not None:  # test hook (e.g. simulator)
        results = _run(nc, in_maps)
    else:
        results = run_bass_kernel_spmd(
            nc, in_maps, core_ids=list(range(cfg.ncores))
        ).results
    out = np.concatenate([results[c]["out"] for c in range(cfg.ncores)], axis=0)
    return out.astype(np.float32)



# revision 10
# speedup vs baseline: 1.3463x; 1.3463x over previous
"""Two-layer GCN (GCNConv x2 + log_softmax) on 8 Trainium2 NeuronCores.

Strategy (graph/data parallel, nodes sharded 8 ways; v2 "gather-x" design):
  - No hidden gather table for layer 1: per-edge messages are the pre-scaled
    input rows xs = dinv*x themselves ([100k, 128] bf16 = 256B rows).  The
    aggregation matmul accumulates acc[in_feat, dst] += X_col^T @ S_col and
    W1 is applied once per 512-dst group AFTER aggregation
    (A @ X) @ W1 == A @ (X @ W1).
  - Self-loops (and the handful of natural src==dst edges) never enter the
    gather stream; their contribution mult[d]*dinv[d]*h[d] is added in the
    epilogues from locally-available rows.
  - Edge slots are bucketed by (512-dst fat group, 25k src window), sorted by
    local dst id, densely packed per core, and padded only to the
    max-over-cores column count (static SPMD program; per-core idx/lid data).
    Columns whose lid range crosses a 128 boundary get one S subcolumn per
    touched 128-window (static union over cores).
  - S columns are built with tensor_tensor is_equal against an iota row using
    a pair-duplicated lid stream so every operand has innermost stride 1 and
    the DVE 2x perf mode applies.
  - Layer 1 accumulates feature-major ([128 in, 512 dst] f32, one PSUM bank
    per fat group); epilogue: W1^T matmul, +self, *dinv, relu+b1, *dinv ->
    t2t; W2 matmul -> node-major h2 staging -> AllGather of compact
    [12500, 40] bf16 slices; local expand into the 256B-row layer-2 table.
  - Layer 2 accumulates node-major ([128 dst, 4x64] f32, half a PSUM bank per
    fat group); all groups are staged to SBUF and one batched epilogue does
    *dinv, +self, +b2 and the log_softmax.
"""

import math
import os
from contextlib import ExitStack
from dataclasses import dataclass

import numpy as np
import ml_dtypes

import concourse.bass as bass
import concourse.tile as tile
from concourse import bacc, mybir
from concourse.bass_utils import run_bass_kernel_spmd

F32 = mybir.dt.float32
BF16 = mybir.dt.bfloat16
I16 = mybir.dt.int16
AF = mybir.ActivationFunctionType
ALU = mybir.AluOpType


@dataclass
class Cfg:
    n: int = 100000        # nodes
    nin: int = 128         # input features
    hid: int = 64          # hidden features
    outf: int = 40         # output features
    ncores: int = 8
    nwin: int = 4          # src windows (int16 idx range)
    g: int = 128           # slot column height / lid window
    gf: int = 512          # fat dst group (PSUM bank granularity)
    chunk_f: int = 4       # fat groups per gather-call chunk
    sub: int = 32          # S subcolumns per build slab

    @property
    def per(self):
        return self.n // self.ncores

    @property
    def win(self):
        return self.n // self.nwin

    @property
    def ngf(self):
        return math.ceil(self.per / self.gf)

    @property
    def nlw(self):
        return self.gf // self.g

    @property
    def nreg(self):
        # 128-node regions covering per (rounded up to fat-group multiples)
        return self.ngf * self.nlw

    @property
    def perp(self):
        return self.nreg * self.g


# ---------------------------------------------------------------- host side


def _preprocess(x, edge_index, W1, b1, W2, b2, cfg: Cfg):
    n, per, g, gf, win = cfg.n, cfg.per, cfg.g, cfg.gf, cfg.win
    nc_, ngf, nwin, nlw = cfg.ncores, cfg.ngf, cfg.nwin, cfg.nlw

    src0 = edge_index[0].astype(np.int64)
    dst0 = edge_index[1].astype(np.int64)

    # degree includes the reference's appended self-loops
    deg = (np.bincount(dst0, minlength=n) + 1).astype(np.float64)
    dinv = (1.0 / np.sqrt(deg)).astype(np.float32)
    # natural self-loops fold into the analytic self term
    mult = np.ones(n, dtype=np.float32)
    selfm = src0 == dst0
    np.add.at(mult, dst0[selfm], 1.0)

    xs = (x * dinv[:, None]).astype(ml_dtypes.bfloat16)  # gather table rows

    cross = ~selfm
    src = src0[cross]
    dst = dst0[cross]

    core = dst // per
    fat = (dst % per) // gf
    wsrc = src // win
    lid = (dst % per) % gf

    # counts per (core, fat, wsrc) -> shared column counts (max over cores)
    counts = np.zeros((nc_, ngf, nwin), dtype=np.int64)
    np.add.at(counts, (core, fat, wsrc), 1)
    cols_fw = np.ceil(counts.max(axis=0) / g).astype(np.int64)  # [ngf, nwin]
    assert (counts > 0).all(), "empty (core,fat,wsrc) bucket"

    # per-core cumulative lid-window counts inside each bucket (for w-sets)
    cnt_lw = np.zeros((nc_, ngf, nwin, nlw), dtype=np.int64)
    np.add.at(cnt_lw, (core, fat, wsrc, lid // g), 1)
    cum_lw = np.zeros((nc_, ngf, nwin, nlw + 1), dtype=np.int64)
    cum_lw[..., 1:] = np.cumsum(cnt_lw, axis=-1)

    # slot layout: for chunk: for wsrc: for fat in chunk: cols_fw[fat,wsrc]*g
    chunks = [
        list(range(k0, min(k0 + cfg.chunk_f, ngf)))
        for k0 in range(0, ngf, cfg.chunk_f)
    ]
    region_off = {}      # (fat, wsrc) -> slot offset
    call_meta = []       # per (chunk, wsrc): (sl0, nsl, [(fat, col0, ncols)])
    off = 0
    for K in chunks:
        for w in range(nwin):
            sl0 = off
            items = []
            for f in K:
                ncol = int(cols_fw[f, w])
                region_off[(f, w)] = off
                items.append((f, (off - sl0) // g, ncol))
                off += ncol * g
            call_meta.append((sl0, off - sl0, items))
    tot_slots = off
    tot_cols = tot_slots // g

    # subcolumn structure: per (fat, wsrc, col): sorted list of lid-windows
    # (union over cores of windows overlapping the column's slot range)
    sub_ws = {}
    real_rows = [min(gf, per - f * gf) for f in range(ngf)]
    nsub_tot = 0
    for K in chunks:
        for w in range(nwin):
            for f in K:
                ncol = int(cols_fw[f, w])
                for j in range(ncol):
                    lo, hi = j * g, j * g + g
                    ws = set()
                    for c in range(nc_):
                        cc = cum_lw[c, f, w]
                        for lw in range(nlw):
                            if cc[lw] < hi and cc[lw + 1] > lo:
                                ws.add(lw)
                    wl = sorted(ws)
                    assert wl, (f, w, j)
                    sub_ws[(f, w, j)] = wl
                    nsub_tot += len(wl)
    # every real region must receive at least one matmul (PSUM init)
    covered = set()
    for (f, w, j), wl in sub_ws.items():
        for lw in wl:
            covered.add((f, lw))
    for f in range(ngf):
        for lw in range(math.ceil(real_rows[f] / g)):
            assert (f, lw) in covered, (f, lw)

    # ---- per-core idx / lidloc arrays ----
    order = np.lexsort((lid, wsrc, fat, core))
    src_s = src[order]
    core_s, fat_s, w_s, lid_s = core[order], fat[order], wsrc[order], lid[order]

    idx_all = np.zeros((nc_, tot_slots), dtype=np.int16)
    lid_all = np.full((nc_, tot_slots), 4 * g, dtype=np.int64)  # pad sentinel
    for c in range(nc_):
        m = core_s == c
        sc_, fc, wc, lc = src_s[m], fat_s[m], w_s[m], lid_s[m]
        key = fc * nwin + wc
        change = np.r_[True, key[1:] != key[:-1]]
        run_id = np.cumsum(change) - 1
        run_start = np.flatnonzero(change)
        rank = np.arange(len(key)) - run_start[run_id]
        base = np.array(
            [region_off[(ff, ww)] for ff, ww in zip(fc[change], wc[change])]
        )
        slot = base[run_id] + rank
        idx_all[c, slot] = (sc_ - wc * win).astype(np.int16)
        lid_all[c, slot] = lc

    # idx wrapped [128, tot_slots//16] (16-partition wrap, replicated x8)
    idx_wrap = np.zeros((nc_, 128, tot_slots // 16), dtype=np.int16)
    for c in range(nc_):
        wrapped = idx_all[c].reshape(-1, 16).T
        idx_wrap[c] = np.tile(wrapped, (8, 1))

    # lid2 stream: per subcol (ordered like the program consumes them):
    # 128 lidloc values pair-duplicated -> [128, 2*nsub_tot]
    lid2 = np.zeros((nc_, 128, 2 * nsub_tot), dtype=ml_dtypes.bfloat16)
    mm_meta = []  # per call: [(colpos_in_call, lw, fat, subidx)]
    sidx = 0
    ci = 0
    for K in chunks:
        for w in range(nwin):
            sl0, nsl, items = call_meta[ci]
            ci += 1
            mlist = []
            for f, col0, ncol in items:
                for j in range(ncol):
                    slot0 = region_off[(f, w)] + j * g
                    col_lids = lid_all[:, slot0 : slot0 + g]  # [nc, 128]
                    for lw in sub_ws[(f, w, j)]:
                        ll = col_lids - lw * g
                        ll = np.where((ll >= 0) & (ll < g), ll, 255)
                        v = ll.astype(ml_dtypes.bfloat16)  # [nc, 128]
                        lid2[:, :, 2 * sidx] = v
                        lid2[:, :, 2 * sidx + 1] = v
                        mlist.append((col0 + j, lw, f, sidx))
                        sidx += 1
            mm_meta.append(mlist)
    assert sidx == nsub_tot

    # ---- small constants ----
    sub = cfg.sub
    iota_rep = np.tile(
        np.tile(np.arange(g, dtype=np.float32), sub), (128, 1)
    ).astype(ml_dtypes.bfloat16)  # [128, sub*g]
    W1bf = W1.astype(ml_dtypes.bfloat16)  # [nin, hid]
    W2p = np.zeros((cfg.hid, cfg.hid), dtype=np.float32)
    W2p[:, : cfg.outf] = W2
    W2bf = W2p.astype(ml_dtypes.bfloat16)
    b1col = b1.reshape(cfg.hid, 1).astype(np.float32)
    b2bc = np.zeros((128, cfg.hid), dtype=np.float32)
    b2bc[:, : cfg.outf] = b2[None, :]

    perp, nreg = cfg.perp, cfg.nreg
    in_maps = []
    for c in range(nc_):
        lo, hi = c * per, (c + 1) * per
        dslice = np.concatenate([dinv[lo:hi], np.ones(perp - per, np.float32)])
        mslice = np.concatenate([mult[lo:hi], np.zeros(perp - per, np.float32)])
        # feature-major own x slice, pre-scaled by dinv*mult (self term)
        xso = np.zeros((cfg.nin, perp), dtype=np.float32)
        xso[:, :per] = (x[lo:hi] * (dinv[lo:hi] * mult[lo:hi])[:, None]).T
        dinv_bc = np.tile(dslice, (cfg.hid, 1)).astype(ml_dtypes.bfloat16)
        # node-major [128, nreg] scale tables for the layer-2 epilogue
        dcol = dslice.reshape(nreg, g).T.copy()
        scol = (dslice * mslice).reshape(nreg, g).T.copy()
        in_maps.append(
            {
                "xs": np.asarray(xs),
                "xsT_own": np.asarray(xso.astype(ml_dtypes.bfloat16)),
                "W1bf": np.asarray(W1bf),
                "W2bf": np.asarray(W2bf),
                "b1col": b1col,
                "b2bc": b2bc,
                "iota": np.asarray(iota_rep),
                "idx": idx_wrap[c],
                "lid2": np.asarray(lid2[c]),
                "dinv_bc": np.asarray(dinv_bc),
                "dinv_col": dcol,
                "self_col": scol,
            }
        )

    sched = dict(
        call_meta=call_meta,
        chunks=chunks,
        mm_meta=mm_meta,
        tot_slots=tot_slots,
        tot_cols=tot_cols,
        nsub_tot=nsub_tot,
    )
    return in_maps, sched


# ---------------------------------------------------------------- device side


def _build(cfg: Cfg, sched) -> bacc.Bacc:
    n, hid, g, gf, nwin, win = cfg.n, cfg.hid, cfg.g, cfg.gf, cfg.nwin, cfg.win
    ngf, per, perp, nreg, nlw = cfg.ngf, cfg.per, cfg.perp, cfg.nreg, cfg.nlw
    nin, of_ = cfg.nin, cfg.outf
    call_meta, chunks, mm_meta = sched["call_meta"], sched["chunks"], sched["mm_meta"]
    tot_slots, nsub_tot = sched["tot_slots"], sched["nsub_tot"]
    max_call_cols = max((m[1] // g for m in call_meta), default=1)

    nc = bacc.Bacc("TRN2", target_bir_lowering=False, debug=False,
                   num_devices=cfg.ncores)

    xs_d = nc.dram_tensor("xs", [n, nin], BF16, kind="ExternalInput").ap()
    xso_d = nc.dram_tensor("xsT_own", [nin, perp], BF16, kind="ExternalInput").ap()
    W1bf = nc.dram_tensor("W1bf", [nin, hid], BF16, kind="ExternalInput").ap()
    W2bf = nc.dram_tensor("W2bf", [hid, hid], BF16, kind="ExternalInput").ap()
    b1col = nc.dram_tensor("b1col", [hid, 1], F32, kind="ExternalInput").ap()
    b2bc = nc.dram_tensor("b2bc", [128, hid], F32, kind="ExternalInput").ap()
    iota_d = nc.dram_tensor("iota", [128, cfg.sub * g], BF16, kind="ExternalInput").ap()
    idx_d = nc.dram_tensor("idx", [128, tot_slots // 16], I16, kind="ExternalInput").ap()
    lid2_d = nc.dram_tensor("lid2", [128, 2 * nsub_tot], BF16, kind="ExternalInput").ap()
    dinvbc_d = nc.dram_tensor("dinv_bc", [hid, perp], BF16, kind="ExternalInput").ap()
    dinvcol_d = nc.dram_tensor("dinv_col", [128, nreg], F32, kind="ExternalInput").ap()
    selfcol_d = nc.dram_tensor("self_col", [128, nreg], F32, kind="ExternalInput").ap()

    out_d = nc.dram_tensor("out", [per, of_], F32, kind="ExternalOutput").ap()

    h2c_b = nc.dram_tensor("h2c_b", [per, of_], BF16).ap()   # AllGather input
    T3c = nc.dram_tensor("T3c", [n, of_], BF16, addr_space="Shared").ap()
    T3 = nc.dram_tensor("T3", [n, nin], BF16).ap()           # 256B-row table

    from concourse import library_config

    with tile.TileContext(nc) as tc, ExitStack() as ctx:
        nc.gpsimd.load_library(library_config.mlp)

        consts = ctx.enter_context(tc.tile_pool(name="consts", bufs=1))
        sb = ctx.enter_context(tc.tile_pool(name="sb", bufs=3))
        subp = ctx.enter_context(tc.tile_pool(name="subp", bufs=2))
        eptmp = ctx.enter_context(tc.tile_pool(name="eptmp", bufs=2))
        fep = ctx.enter_context(tc.tile_pool(name="fep", bufs=1))
        psum_acc = ctx.enter_context(tc.tile_pool(name="psuma", bufs=5, space="PSUM"))
        psum_mm = ctx.enter_context(tc.tile_pool(name="psummm", bufs=2, space="PSUM"))

        # resident constants
        w1_t = consts.tile([nin, hid], BF16)
        nc.sync.dma_start(w1_t[:], W1bf[:, :])
        w2_t = consts.tile([hid, hid], BF16)
        nc.sync.dma_start(w2_t[:], W2bf[:, :])
        b1_t = consts.tile([hid, 1], F32)
        nc.sync.dma_start(b1_t[:], b1col[:, :])
        b2_t = consts.tile([128, hid], F32)
        nc.sync.dma_start(b2_t[:], b2bc[:, :])
        iota_t = consts.tile([128, cfg.sub * g], BF16)
        nc.sync.dma_start(iota_t[:], iota_d[:, :])
        dinvbc_t = consts.tile([hid, perp], BF16)
        nc.sync.dma_start(dinvbc_t[:], dinvbc_d[:, :])
        dinvcol_t = consts.tile([128, nreg], F32)
        nc.sync.dma_start(dinvcol_t[:], dinvcol_d[:, :])
        selfcol_t = consts.tile([128, nreg], F32)
        nc.sync.dma_start(selfcol_t[:], selfcol_d[:, :])
        lid2_t = consts.tile([128, 2 * nsub_tot], BF16)
        nc.sync.dma_start(lid2_t[:], lid2_d[:, :])

        t1own = consts.tile([hid, perp], BF16)      # self rows mult*dinv*h1
        h2stage = consts.tile([128, nreg, hid], BF16)   # local h2 rows (nm)
        acc2stage = consts.tile([128, nreg, hid], F32)  # layer-2 aggregates

        # ---------------- t1own = (mult*dinv*x own)^T @ W1, feature-major
        for f0 in range(0, perp, gf):
            xso_t = sb.tile([nin, gf], BF16, tag="xso")
            nc.sync.dma_start(xso_t[:], xso_d[:, f0 : f0 + gf])
            pt = psum_mm.tile([hid, gf], F32, tag="mm", name="t1own_p")
            nc.tensor.matmul(
                out=pt[:], lhsT=w1_t[:], rhs=xso_t[:],
                start=True, stop=True,
            )
            nc.vector.tensor_copy(t1own[:, f0 : f0 + gf], pt[:])

        # ---------------- aggregation layers
        def agg_layer(layer: int, table_ap):
            ci = 0
            for K in chunks:
                # one PSUM bank per fat group (layer 1) / half bank (layer 2)
                if layer == 1:
                    banks = {
                        f: psum_acc.tile([128, gf], F32, tag="acc", name=f"a1_{f}")
                        for f in K
                    }

                    def acc_ap(f, lw):
                        return banks[f][:, lw * g : (lw + 1) * g]
                else:
                    bt = {}
                    for i in range(0, len(K), 2):
                        t = psum_acc.tile([128, 512], F32, tag="acc",
                                          name=f"a2_{K[i]}")
                        for j, f in enumerate(K[i : i + 2]):
                            bt[f] = (t, j)
                    banks = bt

                    def acc_ap(f, lw):
                        t, j = banks[f]
                        return t[:, (j * nlw + lw) * hid : (j * nlw + lw + 1) * hid]

                # per-bank first/last matmul bookkeeping
                def bank_key(f):
                    return id(banks[f]) if layer == 1 else id(banks[f][0])

                tot_bank = {}
                for w in range(nwin):
                    for cp, lw, f, si in mm_meta[ci + w]:
                        tot_bank[bank_key(f)] = tot_bank.get(bank_key(f), 0) + 1
                emitted = dict.fromkeys(tot_bank, 0)

                for w in range(nwin):
                    sl0, nsl, items = call_meta[ci]
                    mlist = mm_meta[ci]
                    ci += 1
                    cols = nsl // g
                    it = sb.tile([128, max_call_cols * 8], I16, tag="idx")
                    nc.sync.dma_start(
                        it[:, : nsl // 16], idx_d[:, sl0 // 16 : (sl0 + nsl) // 16]
                    )
                    mt = sb.tile([128, max_call_cols, nin], BF16, tag="m")
                    nc.gpsimd.dma_gather(
                        mt[:, :cols, :],
                        table_ap[w * win : (w + 1) * win, :],
                        it[:, : nsl // 16],
                        nsl,
                        nsl,
                        nin,
                        single_packet=False,
                    )
                    # S slabs over this call's subcol range
                    si0 = mlist[0][3]
                    nsub = len(mlist)
                    for s0 in range(0, nsub, cfg.sub):
                        sc = min(cfg.sub, nsub - s0)
                        st_ = subp.tile([128, cfg.sub * g], BF16, tag="sel")
                        l2 = lid2_t[
                            :, 2 * (si0 + s0) : 2 * (si0 + s0 + sc)
                        ].rearrange("p (c t) -> p c t", t=2)
                        nc.vector.tensor_tensor(
                            out=st_[:, : sc * g].rearrange(
                                "p (c r t) -> p c r t", r=g // 2, t=2
                            ),
                            in0=iota_t[:, : sc * g].rearrange(
                                "p (c r t) -> p c r t", r=g // 2, t=2
                            ),
                            in1=l2.unsqueeze(2).broadcast_to([128, sc, g // 2, 2]),
                            op=ALU.is_equal,
                        )
                        for k in range(sc):
                            cp, lw, f, si = mlist[s0 + k]
                            bk = bank_key(f)
                            first = emitted[bk] == 0
                            emitted[bk] += 1
                            last = emitted[bk] == tot_bank[bk]
                            if layer == 1:
                                nc.tensor.matmul(
                                    out=acc_ap(f, lw),
                                    lhsT=mt[:, cp, :],
                                    rhs=st_[:, k * g : (k + 1) * g],
                                    start=first,
                                    stop=last,
                                )
                            else:
                                nc.tensor.matmul(
                                    out=acc_ap(f, lw),
                                    lhsT=st_[:, k * g : (k + 1) * g],
                                    rhs=mt[:, cp, :hid],
                                    start=first,
                                    stop=last,
                                )

                # epilogues for chunk K
                for f in K:
                    c0 = f * gf
                    if layer == 1:
                        accS = eptmp.tile([128, gf], BF16, tag="ep0")
                        nc.vector.tensor_copy(accS[:], banks[f][:])
                        hp = psum_mm.tile([hid, gf], F32, tag="mm", name="h1pre")
                        nc.tensor.matmul(
                            out=hp[:], lhsT=w1_t[:], rhs=accS[:],
                            start=True, stop=True,
                        )
                        t1_ = eptmp.tile([hid, gf], F32, tag="ep1")
                        nc.vector.tensor_add(t1_[:], hp[:], t1own[:, c0 : c0 + gf])
                        t2_ = eptmp.tile([hid, gf], F32, tag="ep2")
                        nc.vector.tensor_mul(
                            t2_[:], t1_[:], dinvbc_t[:, c0 : c0 + gf]
                        )
                        t3_ = eptmp.tile([hid, gf], F32, tag="ep3")
                        nc.scalar.activation(t3_[:], t2_[:], AF.Relu, bias=b1_t[:, :1])
                        t2p = eptmp.tile([hid, gf], BF16, tag="ep4")
                        nc.vector.tensor_mul(
                            t2p[:], t3_[:], dinvbc_t[:, c0 : c0 + gf]
                        )
                        # W2 matmuls -> node-major h2 staging
                        pw = psum_mm.tile([128, nlw * hid], F32, tag="mm", name="pw")
                        for q in range(nlw):
                            nc.tensor.matmul(
                                out=pw[:, q * hid : (q + 1) * hid],
                                lhsT=t2p[:, q * g : (q + 1) * g],
                                rhs=w2_t[:],
                                start=True,
                                stop=True,
                            )
                        nc.vector.tensor_copy(
                            h2stage[:, f * nlw : (f + 1) * nlw, :].rearrange(
                                "p q h -> p (q h)"
                            ),
                            pw[:],
                        )
                    else:
                        t, j = banks[f]
                        nc.vector.tensor_copy(
                            acc2stage[:, f * nlw : (f + 1) * nlw, :].rearrange(
                                "p q h -> p (q h)"
                            ),
                            t[:, j * nlw * hid : (j + 1) * nlw * hid],
                        )

        _phases = int(os.environ.get("GCN_PHASES", "4"))
        if _phases >= 2:
            agg_layer(1, xs_d)

        # ---------------- ship compact h2, AllGather, expand table
        if _phases >= 3:
            nfull = per // g
            rem = per - nfull * g
            nc.sync.dma_start(
                h2c_b[: nfull * g, :].rearrange("(q p) h -> p q h", p=128),
                h2stage[:, :nfull, :of_],
            )
            if rem:
                nc.sync.dma_start(
                    h2c_b[nfull * g : per, :], h2stage[:rem, nfull, :of_]
                )

        if _phases >= 3 and not os.environ.get("GCN_NO_COLL"):
            nc.gpsimd.collective_compute(
                "AllGather",
                ALU.bypass,
                replica_groups=[list(range(cfg.ncores))],
                ins=[h2c_b.opt()],
                outs=[T3c.opt()],
            )
            # expand compact rows into the 256B-row gather table, one src
            # window at a time so layer 2 can start on window 0 early.
            for r0 in range(0, n, win):
                nc.sync.dma_start(T3[r0 : r0 + win, :of_], T3c[r0 : r0 + win, :])

        if _phases >= 4:
            agg_layer(2, T3)

            # ---------------- batched layer-2 epilogue + log_softmax
            # processed in region chunks to bound SBUF temp usage
            fchunk = 10
            for r0 in range(0, nreg, fchunk):
                rn = min(fchunk, nreg - r0)
                a2 = acc2stage[:, r0 : r0 + rn, :]
                nc.vector.tensor_tensor(
                    out=a2,
                    in0=a2,
                    in1=dinvcol_t[:, r0 : r0 + rn]
                    .unsqueeze(2)
                    .broadcast_to([128, rn, hid]),
                    op=ALU.mult,
                )
                o2 = fep.tile([128, fchunk, hid], F32, tag="fe2", name="o2")
                nc.vector.tensor_tensor(
                    out=o2[:, :rn, :],
                    in0=h2stage[:, r0 : r0 + rn, :],
                    in1=selfcol_t[:, r0 : r0 + rn]
                    .unsqueeze(2)
                    .broadcast_to([128, rn, hid]),
                    op=ALU.mult,
                )
                nc.vector.tensor_add(a2, a2, o2[:, :rn, :])
                nc.vector.tensor_tensor(
                    out=a2,
                    in0=a2,
                    in1=b2_t[:].unsqueeze(1).broadcast_to([128, rn, hid]),
                    op=ALU.add,
                )
                nmax = fep.tile([128, fchunk, 1], F32, tag="fm")
                nc.vector.tensor_reduce(
                    nmax[:, :rn, :], a2[:, :, :of_], mybir.AxisListType.X,
                    ALU.max, negate=True,
                )
                sh = fep.tile([128, fchunk, of_], F32, tag="fe3", name="sh")
                nc.vector.tensor_tensor(
                    out=sh[:, :rn, :],
                    in0=a2[:, :, :of_],
                    in1=nmax[:, :rn, :].broadcast_to([128, rn, of_]),
                    op=ALU.add,
                )
                ex = fep.tile([128, fchunk, of_], F32, tag="fe4", name="ex")
                nc.scalar.activation(
                    ex[:, :rn, :].rearrange("p q h -> p (q h)"),
                    sh[:, :rn, :].rearrange("p q h -> p (q h)"),
                    AF.Exp,
                )
                sm = fep.tile([128, fchunk, 1], F32, tag="fs")
                nc.vector.tensor_reduce(
                    sm[:, :rn, :], ex[:, :rn, :], mybir.AxisListType.X, ALU.add
                )
                ls = fep.tile([128, fchunk, 1], F32, tag="fl")
                nc.scalar.activation(
                    ls[:, :rn, :].rearrange("p q h -> p (q h)"),
                    sm[:, :rn, :].rearrange("p q h -> p (q h)"),
                    AF.Ln,
                )
                fin = fep.tile([128, fchunk, of_], F32, tag="fe5", name="fin")
                nc.vector.tensor_tensor(
                    out=fin[:, :rn, :],
                    in0=sh[:, :rn, :],
                    in1=ls[:, :rn, :].broadcast_to([128, rn, of_]),
                    op=ALU.subtract,
                )
                # store: full 128-regions fused, partial region separately
                nf = max(0, min(per // g - r0, rn))
                if nf:
                    nc.sync.dma_start(
                        out_d[r0 * g : (r0 + nf) * g, :].rearrange(
                            "(q p) f -> p q f", p=128
                        ),
                        fin[:, :nf, :],
                    )
                pi = per // g  # partial region index
                if r0 <= pi < r0 + rn and per % g:
                    nc.sync.dma_start(
                        out_d[pi * g : per, :], fin[: per % g, pi - r0, :]
                    )

    nc.compile()
    return nc


# ---------------------------------------------------------------- entry


def kernel(x, edge_index, W1, b1, W2, b2, cfg: Cfg | None = None, _run=None):
    cfg = cfg or Cfg()
    in_maps, sched = _preprocess(
        np.asarray(x), np.asarray(edge_index), np.asarray(W1), np.asarray(b1),
        np.asarray(W2), np.asarray(b2), cfg
    )
    nc = _build(cfg, sched)
    if _run is not None:  # test hook (e.g. simulator)
        results = _run(nc, in_maps)
    else:
        results = run_bass_kernel_spmd(
            nc, in_maps, core_ids=list(range(cfg.ncores))
        ).results
    out = np.concatenate([results[c]["out"] for c in range(cfg.ncores)], axis=0)
    return out.astype(np.float32)


# revision 12
# speedup vs baseline: 1.4172x; 1.0526x over previous
"""Two-layer GCN (GCNConv x2 + log_softmax) on 8 Trainium2 NeuronCores.

Strategy (graph/data parallel, nodes sharded 8 ways; v2 "gather-x" design):
  - No hidden gather table for layer 1: per-edge messages are the pre-scaled
    input rows xs = dinv*x themselves ([100k, 128] bf16 = 256B rows).  The
    aggregation matmul accumulates acc[in_feat, dst] += X_col^T @ S_col and
    W1 is applied once per 512-dst group AFTER aggregation
    (A @ X) @ W1 == A @ (X @ W1).
  - Self-loops (and the handful of natural src==dst edges) never enter the
    gather stream; their contribution mult[d]*dinv[d]*h[d] is added in the
    epilogues from locally-available rows.
  - Edge slots are bucketed by (512-dst fat group, 25k src window), sorted by
    local dst id, densely packed per core, and padded only to the
    max-over-cores column count (static SPMD program; per-core idx/lid data).
    Columns whose lid range crosses a 128 boundary get one S subcolumn per
    touched 128-window (static union over cores).
  - S columns are built with tensor_tensor is_equal against an iota row using
    a pair-duplicated lid stream so every operand has innermost stride 1 and
    the DVE 2x perf mode applies.
  - Layer 1 accumulates feature-major ([128 in, 512 dst] f32, one PSUM bank
    per fat group); epilogue: W1^T matmul, +self, *dinv, relu+b1, *dinv ->
    t2t; W2 matmul -> node-major h2 staging -> AllGather of compact
    [12500, 40] bf16 slices; local expand into the 256B-row layer-2 table.
  - Layer 2 accumulates node-major ([128 dst, 4x64] f32, half a PSUM bank per
    fat group); all groups are staged to SBUF and one batched epilogue does
    *dinv, +self, +b2 and the log_softmax.
"""

import math
import os
from contextlib import ExitStack
from dataclasses import dataclass

import numpy as np
import ml_dtypes

import concourse.bass as bass
import concourse.tile as tile
from concourse import bacc, mybir
from concourse.bass_utils import run_bass_kernel_spmd

F32 = mybir.dt.float32
BF16 = mybir.dt.bfloat16
I16 = mybir.dt.int16
AF = mybir.ActivationFunctionType
ALU = mybir.AluOpType


@dataclass
class Cfg:
    n: int = 100000        # nodes
    nin: int = 128         # input features
    hid: int = 64          # hidden features
    outf: int = 40         # output features
    ncores: int = 8
    nwin: int = 4          # src windows (int16 idx range)
    g: int = 128           # slot column height / lid window
    gf: int = 512          # fat dst group (PSUM bank granularity)
    chunk_f: int = 4       # fat groups per gather-call chunk
    sub: int = 32          # S subcolumns per build slab

    @property
    def per(self):
        return self.n // self.ncores

    @property
    def win(self):
        return self.n // self.nwin

    @property
    def ngf(self):
        return math.ceil(self.per / self.gf)

    @property
    def nlw(self):
        return self.gf // self.g

    @property
    def nreg(self):
        # 128-node regions covering per (rounded up to fat-group multiples)
        return self.ngf * self.nlw

    @property
    def perp(self):
        return self.nreg * self.g


# ---------------------------------------------------------------- host side


def _preprocess(x, edge_index, W1, b1, W2, b2, cfg: Cfg):
    n, per, g, gf, win = cfg.n, cfg.per, cfg.g, cfg.gf, cfg.win
    nc_, ngf, nwin, nlw = cfg.ncores, cfg.ngf, cfg.nwin, cfg.nlw

    src0 = edge_index[0].astype(np.int64)
    dst0 = edge_index[1].astype(np.int64)

    # degree includes the reference's appended self-loops
    deg = (np.bincount(dst0, minlength=n) + 1).astype(np.float64)
    dinv = (1.0 / np.sqrt(deg)).astype(np.float32)
    # natural self-loops fold into the analytic self term
    mult = np.ones(n, dtype=np.float32)
    selfm = src0 == dst0
    np.add.at(mult, dst0[selfm], 1.0)

    xs = (x * dinv[:, None]).astype(ml_dtypes.bfloat16)  # gather table rows

    cross = ~selfm
    src = src0[cross]
    dst = dst0[cross]

    core = dst // per
    fat = (dst % per) // gf
    wsrc = src // win
    lid = (dst % per) % gf

    # counts per (core, fat, wsrc) -> shared column counts (max over cores)
    counts = np.zeros((nc_, ngf, nwin), dtype=np.int64)
    np.add.at(counts, (core, fat, wsrc), 1)
    cols_fw = np.ceil(counts.max(axis=0) / g).astype(np.int64)  # [ngf, nwin]
    assert (counts > 0).all(), "empty (core,fat,wsrc) bucket"

    # per-core cumulative lid-window counts inside each bucket (for w-sets)
    cnt_lw = np.zeros((nc_, ngf, nwin, nlw), dtype=np.int64)
    np.add.at(cnt_lw, (core, fat, wsrc, lid // g), 1)
    cum_lw = np.zeros((nc_, ngf, nwin, nlw + 1), dtype=np.int64)
    cum_lw[..., 1:] = np.cumsum(cnt_lw, axis=-1)

    # slot layout: for chunk: for wsrc: for fat in chunk: cols_fw[fat,wsrc]*g
    chunks = [
        list(range(k0, min(k0 + cfg.chunk_f, ngf)))
        for k0 in range(0, ngf, cfg.chunk_f)
    ]
    region_off = {}      # (fat, wsrc) -> slot offset
    call_meta = []       # per (chunk, wsrc): (sl0, nsl, [(fat, col0, ncols)])
    off = 0
    for K in chunks:
        for w in range(nwin):
            sl0 = off
            items = []
            for f in K:
                ncol = int(cols_fw[f, w])
                region_off[(f, w)] = off
                items.append((f, (off - sl0) // g, ncol))
                off += ncol * g
            call_meta.append((sl0, off - sl0, items))
    tot_slots = off
    tot_cols = tot_slots // g

    # subcolumn structure: per (fat, wsrc, col): sorted list of lid-windows
    # (union over cores of windows overlapping the column's slot range)
    sub_ws = {}
    real_rows = [min(gf, per - f * gf) for f in range(ngf)]
    nsub_tot = 0
    for K in chunks:
        for w in range(nwin):
            for f in K:
                ncol = int(cols_fw[f, w])
                for j in range(ncol):
                    lo, hi = j * g, j * g + g
                    ws = set()
                    for c in range(nc_):
                        cc = cum_lw[c, f, w]
                        for lw in range(nlw):
                            if cc[lw] < hi and cc[lw + 1] > lo:
                                ws.add(lw)
                    wl = sorted(ws)
                    assert wl, (f, w, j)
                    sub_ws[(f, w, j)] = wl
                    nsub_tot += len(wl)
    # every real region must receive at least one matmul (PSUM init)
    covered = set()
    for (f, w, j), wl in sub_ws.items():
        for lw in wl:
            covered.add((f, lw))
    for f in range(ngf):
        for lw in range(math.ceil(real_rows[f] / g)):
            assert (f, lw) in covered, (f, lw)

    # ---- per-core idx / lidloc arrays ----
    order = np.lexsort((lid, wsrc, fat, core))
    src_s = src[order]
    core_s, fat_s, w_s, lid_s = core[order], fat[order], wsrc[order], lid[order]

    idx_all = np.zeros((nc_, tot_slots), dtype=np.int16)
    lid_all = np.full((nc_, tot_slots), 4 * g, dtype=np.int64)  # pad sentinel
    for c in range(nc_):
        m = core_s == c
        sc_, fc, wc, lc = src_s[m], fat_s[m], w_s[m], lid_s[m]
        key = fc * nwin + wc
        change = np.r_[True, key[1:] != key[:-1]]
        run_id = np.cumsum(change) - 1
        run_start = np.flatnonzero(change)
        rank = np.arange(len(key)) - run_start[run_id]
        base = np.array(
            [region_off[(ff, ww)] for ff, ww in zip(fc[change], wc[change])]
        )
        slot = base[run_id] + rank
        idx_all[c, slot] = (sc_ - wc * win).astype(np.int16)
        lid_all[c, slot] = lc

    # idx wrapped [128, tot_slots//16] (16-partition wrap, replicated x8)
    idx_wrap = np.zeros((nc_, 128, tot_slots // 16), dtype=np.int16)
    for c in range(nc_):
        wrapped = idx_all[c].reshape(-1, 16).T
        idx_wrap[c] = np.tile(wrapped, (8, 1))

    # lid2 stream: per subcol (ordered like the program consumes them):
    # 128 lidloc values pair-duplicated -> [128, 2*nsub_tot]
    lid2 = np.zeros((nc_, 128, 2 * nsub_tot), dtype=ml_dtypes.bfloat16)
    mm_meta = []  # per call: [(colpos_in_call, lw, fat, subidx)]
    sidx = 0
    ci = 0
    for K in chunks:
        for w in range(nwin):
            sl0, nsl, items = call_meta[ci]
            ci += 1
            mlist = []
            for f, col0, ncol in items:
                for j in range(ncol):
                    slot0 = region_off[(f, w)] + j * g
                    col_lids = lid_all[:, slot0 : slot0 + g]  # [nc, 128]
                    for lw in sub_ws[(f, w, j)]:
                        ll = col_lids - lw * g
                        ll = np.where((ll >= 0) & (ll < g), ll, 255)
                        v = ll.astype(ml_dtypes.bfloat16)  # [nc, 128]
                        lid2[:, :, 2 * sidx] = v
                        lid2[:, :, 2 * sidx + 1] = v
                        mlist.append((col0 + j, lw, f, sidx))
                        sidx += 1
            mm_meta.append(mlist)
    assert sidx == nsub_tot

    # ---- small constants ----
    sub = cfg.sub
    iota_rep = np.tile(
        np.tile(np.arange(g, dtype=np.float32), sub), (128, 1)
    ).astype(ml_dtypes.bfloat16)  # [128, sub*g]
    W1bf = W1.astype(ml_dtypes.bfloat16)  # [nin, hid]
    W2p = np.zeros((cfg.hid, cfg.hid), dtype=np.float32)
    W2p[:, : cfg.outf] = W2
    W2bf = W2p.astype(ml_dtypes.bfloat16)
    b1col = b1.reshape(cfg.hid, 1).astype(np.float32)
    b2bc = np.zeros((128, cfg.hid), dtype=np.float32)
    b2bc[:, : cfg.outf] = b2[None, :]
    hasb1 = bool(np.any(b1))
    hasb2 = bool(np.any(b2))
    ident64 = np.eye(cfg.hid, dtype=np.float32).astype(ml_dtypes.bfloat16)

    perp, nreg = cfg.perp, cfg.nreg
    in_maps = []
    for c in range(nc_):
        lo, hi = c * per, (c + 1) * per
        dslice = np.concatenate([dinv[lo:hi], np.ones(perp - per, np.float32)])
        mslice = np.concatenate([mult[lo:hi], np.zeros(perp - per, np.float32)])
        # feature-major own x slice, pre-scaled by dinv*mult (self term)
        xso = np.zeros((cfg.nin, perp), dtype=np.float32)
        xso[:, :per] = (x[lo:hi] * (dinv[lo:hi] * mult[lo:hi])[:, None]).T
        dbase = dslice if hasb1 else dslice * dslice
        dinv_bc = np.tile(dbase, (cfg.hid, 1)).astype(ml_dtypes.bfloat16)
        # node-major [128, nreg] scale tables for the layer-2 epilogue
        dcol = dslice.reshape(nreg, g).T.copy()
        scol = (dslice * mslice).reshape(nreg, g).T.copy()
        in_maps.append(
            {
                "xs": np.asarray(xs),
                "xsT_own": np.asarray(xso.astype(ml_dtypes.bfloat16)),
                "W1bf": np.asarray(W1bf),
                "W2bf": np.asarray(W2bf),
                "b1col": b1col,
                "b2bc": b2bc,
                "iota": np.asarray(iota_rep),
                "ident64": np.asarray(ident64),
                "idx": idx_wrap[c],
                "lid2": np.asarray(lid2[c]),
                "dinv_bc": np.asarray(dinv_bc),
                "dinv_col": dcol,
                "self_col": scol,
            }
        )

    sched = dict(
        call_meta=call_meta,
        chunks=chunks,
        mm_meta=mm_meta,
        tot_slots=tot_slots,
        tot_cols=tot_cols,
        nsub_tot=nsub_tot,
        hasb1=hasb1,
        hasb2=hasb2,
    )
    return in_maps, sched


# ---------------------------------------------------------------- device side


def _build(cfg: Cfg, sched) -> bacc.Bacc:
    n, hid, g, gf, nwin, win = cfg.n, cfg.hid, cfg.g, cfg.gf, cfg.nwin, cfg.win
    ngf, per, perp, nreg, nlw = cfg.ngf, cfg.per, cfg.perp, cfg.nreg, cfg.nlw
    nin, of_ = cfg.nin, cfg.outf
    call_meta, chunks, mm_meta = sched["call_meta"], sched["chunks"], sched["mm_meta"]
    hasb1, hasb2 = sched["hasb1"], sched["hasb2"]
    tot_slots, nsub_tot = sched["tot_slots"], sched["nsub_tot"]
    max_call_cols = max((m[1] // g for m in call_meta), default=1)

    nc = bacc.Bacc("TRN2", target_bir_lowering=False, debug=False,
                   num_devices=cfg.ncores)

    xs_d = nc.dram_tensor("xs", [n, nin], BF16, kind="ExternalInput").ap()
    xso_d = nc.dram_tensor("xsT_own", [nin, perp], BF16, kind="ExternalInput").ap()
    W1bf = nc.dram_tensor("W1bf", [nin, hid], BF16, kind="ExternalInput").ap()
    W2bf = nc.dram_tensor("W2bf", [hid, hid], BF16, kind="ExternalInput").ap()
    b1col = nc.dram_tensor("b1col", [hid, 1], F32, kind="ExternalInput").ap()
    b2bc = nc.dram_tensor("b2bc", [128, hid], F32, kind="ExternalInput").ap()
    iota_d = nc.dram_tensor("iota", [128, cfg.sub * g], BF16, kind="ExternalInput").ap()
    ident_d = nc.dram_tensor("ident64", [hid, hid], BF16, kind="ExternalInput").ap()
    idx_d = nc.dram_tensor("idx", [128, tot_slots // 16], I16, kind="ExternalInput").ap()
    lid2_d = nc.dram_tensor("lid2", [128, 2 * nsub_tot], BF16, kind="ExternalInput").ap()
    dinvbc_d = nc.dram_tensor("dinv_bc", [hid, perp], BF16, kind="ExternalInput").ap()
    dinvcol_d = nc.dram_tensor("dinv_col", [128, nreg], F32, kind="ExternalInput").ap()
    selfcol_d = nc.dram_tensor("self_col", [128, nreg], F32, kind="ExternalInput").ap()

    out_d = nc.dram_tensor("out", [per, of_], F32, kind="ExternalOutput").ap()

    h2c_b = nc.dram_tensor("h2c_b", [per, of_], BF16).ap()   # AllGather input
    T3c = nc.dram_tensor("T3c", [n, of_], BF16, addr_space="Shared").ap()
    T3 = nc.dram_tensor("T3", [n, nin], BF16).ap()           # 256B-row table

    from concourse import library_config

    with tile.TileContext(nc) as tc, ExitStack() as ctx:
        nc.gpsimd.load_library(library_config.mlp)

        consts = ctx.enter_context(tc.tile_pool(name="consts", bufs=1))
        sb = ctx.enter_context(tc.tile_pool(name="sb", bufs=3))
        subp = ctx.enter_context(tc.tile_pool(name="subp", bufs=2))
        eptmp = ctx.enter_context(tc.tile_pool(name="eptmp", bufs=2))
        fep = ctx.enter_context(tc.tile_pool(name="fep", bufs=1))
        psum_acc = ctx.enter_context(tc.tile_pool(name="psuma", bufs=5, space="PSUM"))
        psum_mm = ctx.enter_context(tc.tile_pool(name="psummm", bufs=2, space="PSUM"))

        # resident constants
        w1_t = consts.tile([nin, hid], BF16)
        nc.sync.dma_start(w1_t[:], W1bf[:, :])
        w2_t = consts.tile([hid, hid], BF16)
        nc.sync.dma_start(w2_t[:], W2bf[:, :])
        b1_t = consts.tile([hid, 1], F32)
        nc.sync.dma_start(b1_t[:], b1col[:, :])
        b2_t = consts.tile([128, hid], F32)
        nc.sync.dma_start(b2_t[:], b2bc[:, :])
        iota_t = consts.tile([128, cfg.sub * g], BF16)
        nc.sync.dma_start(iota_t[:], iota_d[:, :])
        ident_t = consts.tile([hid, hid], BF16)
        nc.sync.dma_start(ident_t[:], ident_d[:, :])
        dinvbc_t = consts.tile([hid, perp], BF16)
        nc.sync.dma_start(dinvbc_t[:], dinvbc_d[:, :])
        dinvcol_t = consts.tile([128, nreg], F32)
        nc.sync.dma_start(dinvcol_t[:], dinvcol_d[:, :])
        selfcol_t = consts.tile([128, nreg], F32)
        nc.sync.dma_start(selfcol_t[:], selfcol_d[:, :])
        lid2_t = consts.tile([128, 2 * nsub_tot], BF16)
        nc.sync.dma_start(lid2_t[:], lid2_d[:, :])

        t1own = consts.tile([hid, perp], BF16)      # self rows mult*dinv*h1
        h2stage = consts.tile([128, nreg, hid], BF16)   # local h2 rows (nm)
        acc2stage = consts.tile([128, nreg, hid], F32)  # layer-2 aggregates

        # ---------------- t1own = (mult*dinv*x own)^T @ W1, feature-major
        for f0 in range(0, perp, gf):
            xso_t = sb.tile([nin, gf], BF16, tag="xso")
            nc.sync.dma_start(xso_t[:], xso_d[:, f0 : f0 + gf])
            pt = psum_mm.tile([hid, gf], F32, tag="mm", name="t1own_p")
            nc.tensor.matmul(
                out=pt[:], lhsT=w1_t[:], rhs=xso_t[:],
                start=True, stop=True,
            )
            nc.vector.tensor_copy(t1own[:, f0 : f0 + gf], pt[:])

        # ---------------- batched layer-2 epilogue + log_softmax (per
        # region chunk, interleaved with layer-2 aggregation)
        def final_ep(r0, rn):
            a2 = acc2stage[:, r0 : r0 + rn, :]
            nc.vector.tensor_tensor(
                out=a2,
                in0=a2,
                in1=dinvcol_t[:, r0 : r0 + rn]
                .unsqueeze(2)
                .broadcast_to([128, rn, hid]),
                op=ALU.mult,
            )
            o2 = fep.tile([128, rn, hid], F32, tag="fe2", name="o2")
            nc.vector.tensor_tensor(
                out=o2[:],
                in0=h2stage[:, r0 : r0 + rn, :],
                in1=selfcol_t[:, r0 : r0 + rn]
                .unsqueeze(2)
                .broadcast_to([128, rn, hid]),
                op=ALU.mult,
            )
            nc.vector.tensor_add(a2, a2, o2[:])
            if hasb2:
                nc.vector.tensor_tensor(
                    out=a2,
                    in0=a2,
                    in1=b2_t[:].unsqueeze(1).broadcast_to([128, rn, hid]),
                    op=ALU.add,
                )
            nmax = fep.tile([128, rn, 1], F32, tag="fm")
            nc.vector.tensor_reduce(
                nmax[:], a2[:, :, :of_], mybir.AxisListType.X,
                ALU.max, negate=True,
            )
            sh = fep.tile([128, rn, of_], F32, tag="fe3", name="sh")
            nc.vector.tensor_tensor(
                out=sh[:],
                in0=a2[:, :, :of_],
                in1=nmax[:].broadcast_to([128, rn, of_]),
                op=ALU.add,
            )
            ex = fep.tile([128, rn, of_], F32, tag="fe4", name="ex")
            nc.scalar.activation(
                ex[:].rearrange("p q h -> p (q h)"),
                sh[:].rearrange("p q h -> p (q h)"),
                AF.Exp,
            )
            sm = fep.tile([128, rn, 1], F32, tag="fs")
            nc.vector.tensor_reduce(
                sm[:], ex[:], mybir.AxisListType.X, ALU.add
            )
            ls = fep.tile([128, rn, 1], F32, tag="fl")
            nc.scalar.activation(
                ls[:].rearrange("p q h -> p (q h)"),
                sm[:].rearrange("p q h -> p (q h)"),
                AF.Ln,
            )
            fin = fep.tile([128, rn, of_], F32, tag="fe5", name="fin")
            nc.vector.tensor_tensor(
                out=fin[:],
                in0=sh[:],
                in1=ls[:].broadcast_to([128, rn, of_]),
                op=ALU.subtract,
            )
            # store: full 128-regions fused, partial region separately
            nf = max(0, min(per // g - r0, rn))
            if nf:
                nc.sync.dma_start(
                    out_d[r0 * g : (r0 + nf) * g, :].rearrange(
                        "(q p) f -> p q f", p=128
                    ),
                    fin[:, :nf, :],
                )
            pi = per // g  # partial region index
            if r0 <= pi < r0 + rn and per % g:
                nc.sync.dma_start(
                    out_d[pi * g : per, :], fin[: per % g, pi - r0, :]
                )

        # ---------------- aggregation layers
        def agg_layer(layer: int, table_ap):
            ci = 0
            for K in chunks:
                # one PSUM bank per fat group (layer 1) / half bank (layer 2)
                if layer == 1:
                    banks = {
                        f: psum_acc.tile([128, gf], F32, tag="acc", name=f"a1_{f}")
                        for f in K
                    }

                    def acc_ap(f, lw):
                        return banks[f][:, lw * g : (lw + 1) * g]
                else:
                    bt = {}
                    for i in range(0, len(K), 2):
                        t = psum_acc.tile([128, 512], F32, tag="acc",
                                          name=f"a2_{K[i]}")
                        for j, f in enumerate(K[i : i + 2]):
                            bt[f] = (t, j)
                    banks = bt

                    def acc_ap(f, lw):
                        t, j = banks[f]
                        return t[:, (j * nlw + lw) * hid : (j * nlw + lw + 1) * hid]

                # per-bank first/last matmul bookkeeping
                def bank_key(f):
                    return id(banks[f]) if layer == 1 else id(banks[f][0])

                tot_bank = {}
                for w in range(nwin):
                    for cp, lw, f, si in mm_meta[ci + w]:
                        tot_bank[bank_key(f)] = tot_bank.get(bank_key(f), 0) + 1
                emitted = dict.fromkeys(tot_bank, 0)

                for w in range(nwin):
                    sl0, nsl, items = call_meta[ci]
                    mlist = mm_meta[ci]
                    ci += 1
                    cols = nsl // g
                    it = sb.tile([128, max_call_cols * 8], I16, tag="idx")
                    nc.sync.dma_start(
                        it[:, : nsl // 16], idx_d[:, sl0 // 16 : (sl0 + nsl) // 16]
                    )
                    mt = sb.tile([128, max_call_cols, nin], BF16, tag="m")
                    nc.gpsimd.dma_gather(
                        mt[:, :cols, :],
                        table_ap[w * win : (w + 1) * win, :],
                        it[:, : nsl // 16],
                        nsl,
                        nsl,
                        nin,
                        single_packet=False,
                    )
                    # S slabs over this call's subcol range
                    si0 = mlist[0][3]
                    nsub = len(mlist)
                    for s0 in range(0, nsub, cfg.sub):
                        sc = min(cfg.sub, nsub - s0)
                        st_ = subp.tile([128, cfg.sub * g], BF16, tag="sel")
                        l2 = lid2_t[
                            :, 2 * (si0 + s0) : 2 * (si0 + s0 + sc)
                        ].rearrange("p (c t) -> p c t", t=2)
                        nc.vector.tensor_tensor(
                            out=st_[:, : sc * g].rearrange(
                                "p (c r t) -> p c r t", r=g // 2, t=2
                            ),
                            in0=iota_t[:, : sc * g].rearrange(
                                "p (c r t) -> p c r t", r=g // 2, t=2
                            ),
                            in1=l2.unsqueeze(2).broadcast_to([128, sc, g // 2, 2]),
                            op=ALU.is_equal,
                        )
                        for k in range(sc):
                            cp, lw, f, si = mlist[s0 + k]
                            bk = bank_key(f)
                            first = emitted[bk] == 0
                            emitted[bk] += 1
                            last = emitted[bk] == tot_bank[bk]
                            if layer == 1:
                                nc.tensor.matmul(
                                    out=acc_ap(f, lw),
                                    lhsT=mt[:, cp, :],
                                    rhs=st_[:, k * g : (k + 1) * g],
                                    start=first,
                                    stop=last,
                                )
                            else:
                                nc.tensor.matmul(
                                    out=acc_ap(f, lw),
                                    lhsT=st_[:, k * g : (k + 1) * g],
                                    rhs=mt[:, cp, :hid],
                                    start=first,
                                    stop=last,
                                )

                # epilogues for chunk K
                for f in K:
                    c0 = f * gf
                    if layer == 1:
                        accS = eptmp.tile([128, gf], BF16, tag="ep0")
                        nc.vector.tensor_copy(accS[:], banks[f][:])
                        hp = psum_mm.tile([hid, gf], F32, tag="mm", name="h1pre")
                        nc.tensor.matmul(
                            out=hp[:], lhsT=w1_t[:], rhs=accS[:],
                            start=True, stop=False,
                        )
                        nc.tensor.matmul(
                            out=hp[:], lhsT=ident_t[:],
                            rhs=t1own[:, c0 : c0 + gf],
                            start=False, stop=True,
                        )
                        if hasb1:
                            t2_ = eptmp.tile([hid, gf], F32, tag="ep2")
                            nc.vector.tensor_mul(
                                t2_[:], hp[:], dinvbc_t[:, c0 : c0 + gf]
                            )
                            t3_ = eptmp.tile([hid, gf], F32, tag="ep3")
                            nc.scalar.activation(
                                t3_[:], t2_[:], AF.Relu, bias=b1_t[:, :1]
                            )
                            t2p = eptmp.tile([hid, gf], BF16, tag="ep4")
                            nc.vector.tensor_mul(
                                t2p[:], t3_[:], dinvbc_t[:, c0 : c0 + gf]
                            )
                        else:
                            # b1 == 0: relu(dinv*u)*dinv == relu(u)*dinv^2
                            # (dinvbc_t holds dinv^2 in this mode)
                            t3_ = eptmp.tile([hid, gf], BF16, tag="ep3")
                            nc.scalar.activation(t3_[:], hp[:], AF.Relu)
                            t2p = eptmp.tile([hid, gf], BF16, tag="ep4")
                            nc.vector.tensor_mul(
                                t2p[:], t3_[:], dinvbc_t[:, c0 : c0 + gf]
                            )
                        # W2 matmuls -> node-major h2 staging
                        pw = psum_mm.tile([128, nlw * hid], F32, tag="mm", name="pw")
                        for q in range(nlw):
                            nc.tensor.matmul(
                                out=pw[:, q * hid : (q + 1) * hid],
                                lhsT=t2p[:, q * g : (q + 1) * g],
                                rhs=w2_t[:],
                                start=True,
                                stop=True,
                            )
                        nc.vector.tensor_copy(
                            h2stage[:, f * nlw : (f + 1) * nlw, :].rearrange(
                                "p q h -> p (q h)"
                            ),
                            pw[:],
                        )
                    else:
                        t, j = banks[f]
                        nc.vector.tensor_copy(
                            acc2stage[:, f * nlw : (f + 1) * nlw, :].rearrange(
                                "p q h -> p (q h)"
                            ),
                            t[:, j * nlw * hid : (j + 1) * nlw * hid],
                        )
                if layer == 2:
                    final_ep(K[0] * nlw, len(K) * nlw)

        _phases = int(os.environ.get("GCN_PHASES", "4"))
        if _phases >= 2:
            agg_layer(1, xs_d)

        # ---------------- ship compact h2, AllGather, expand table
        if _phases >= 3:
            nfull = per // g
            rem = per - nfull * g
            nc.sync.dma_start(
                h2c_b[: nfull * g, :].rearrange("(q p) h -> p q h", p=128),
                h2stage[:, :nfull, :of_],
            )
            if rem:
                nc.sync.dma_start(
                    h2c_b[nfull * g : per, :], h2stage[:rem, nfull, :of_]
                )

        if _phases >= 3 and not os.environ.get("GCN_NO_COLL"):
            nc.gpsimd.collective_compute(
                "AllGather",
                ALU.bypass,
                replica_groups=[list(range(cfg.ncores))],
                ins=[h2c_b.opt()],
                outs=[T3c.opt()],
            )
            # expand compact rows into the 256B-row gather table, one src
            # window at a time so layer 2 can start on window 0 early.
            for r0 in range(0, n, win):
                nc.sync.dma_start(T3[r0 : r0 + win, :of_], T3c[r0 : r0 + win, :])

        if _phases >= 4:
            agg_layer(2, T3)

    nc.compile()
    return nc


# ---------------------------------------------------------------- entry


def kernel(x, edge_index, W1, b1, W2, b2, cfg: Cfg | None = None, _run=None):
    cfg = cfg or Cfg()
    in_maps, sched = _preprocess(
        np.asarray(x), np.asarray(edge_index), np.asarray(W1), np.asarray(b1),
        np.asarray(W2), np.asarray(b2), cfg
    )
    nc = _build(cfg, sched)
    if _run is not None:  # test hook (e.g. simulator)
        results = _run(nc, in_maps)
    else:
        results = run_bass_kernel_spmd(
            nc, in_maps, core_ids=list(range(cfg.ncores))
        ).results
    out = np.concatenate([results[c]["out"] for c in range(cfg.ncores)], axis=0)
    return out.astype(np.float32)


# revision 15
# speedup vs baseline: 1.4883x; 1.0502x over previous
"""Two-layer GCN (GCNConv x2 + log_softmax) on 8 Trainium2 NeuronCores.

Strategy (graph/data parallel, nodes sharded 8 ways; v2 "gather-x" design):
  - No hidden gather table for layer 1: per-edge messages are the pre-scaled
    input rows xs = dinv*x themselves ([100k, 128] bf16 = 256B rows).  The
    aggregation matmul accumulates acc[in_feat, dst] += X_col^T @ S_col and
    W1 is applied once per 512-dst group AFTER aggregation
    (A @ X) @ W1 == A @ (X @ W1).
  - Self-loops (and the handful of natural src==dst edges) never enter the
    gather stream; their contribution mult[d]*dinv[d]*h[d] is added in the
    epilogues from locally-available rows.
  - Edge slots are bucketed by (512-dst fat group, 25k src window), sorted by
    local dst id, densely packed per core, and padded only to the
    max-over-cores column count (static SPMD program; per-core idx/lid data).
    Columns whose lid range crosses a 128 boundary get one S subcolumn per
    touched 128-window (static union over cores).
  - S columns are built with tensor_tensor is_equal against an iota row using
    a pair-duplicated lid stream so every operand has innermost stride 1 and
    the DVE 2x perf mode applies.
  - Layer 1 accumulates feature-major ([128 in, 512 dst] f32, one PSUM bank
    per fat group); epilogue: W1^T matmul, +self, *dinv, relu+b1, *dinv ->
    t2t; W2 matmul -> node-major h2 staging -> AllGather of compact
    [12500, 40] bf16 slices; local expand into the 256B-row layer-2 table.
  - Layer 2 accumulates node-major ([128 dst, 4x64] f32, half a PSUM bank per
    fat group); all groups are staged to SBUF and one batched epilogue does
    *dinv, +self, +b2 and the log_softmax.
"""

import math
import os
from contextlib import ExitStack
from dataclasses import dataclass

import numpy as np
import ml_dtypes

import concourse.bass as bass
import concourse.tile as tile
from concourse import bacc, mybir
from concourse.bass_utils import run_bass_kernel_spmd

F32 = mybir.dt.float32
BF16 = mybir.dt.bfloat16
I16 = mybir.dt.int16
AF = mybir.ActivationFunctionType
ALU = mybir.AluOpType


@dataclass
class Cfg:
    n: int = 100000        # nodes
    nin: int = 128         # input features
    hid: int = 64          # hidden features
    outf: int = 40         # output features
    ncores: int = 8
    nwin: int = 4          # src windows (int16 idx range)
    g: int = 128           # slot column height / lid window
    gf: int = 512          # fat dst group (PSUM bank granularity)
    chunk_f: int = 4       # fat groups per gather-call chunk
    sub: int = 24          # S subcolumns per build slab

    @property
    def per(self):
        return self.n // self.ncores

    @property
    def win(self):
        return self.n // self.nwin

    @property
    def ngf(self):
        return math.ceil(self.per / self.gf)

    @property
    def nlw(self):
        return self.gf // self.g

    @property
    def nreg(self):
        # 128-node regions covering per (rounded up to fat-group multiples)
        return self.ngf * self.nlw

    @property
    def perp(self):
        return self.nreg * self.g


# ---------------------------------------------------------------- host side


def _preprocess(x, edge_index, W1, b1, W2, b2, cfg: Cfg):
    n, per, g, gf, win = cfg.n, cfg.per, cfg.g, cfg.gf, cfg.win
    nc_, ngf, nwin, nlw = cfg.ncores, cfg.ngf, cfg.nwin, cfg.nlw

    src0 = edge_index[0].astype(np.int64)
    dst0 = edge_index[1].astype(np.int64)

    # degree includes the reference's appended self-loops
    deg = (np.bincount(dst0, minlength=n) + 1).astype(np.float64)
    dinv = (1.0 / np.sqrt(deg)).astype(np.float32)
    # natural self-loops fold into the analytic self term
    mult = np.ones(n, dtype=np.float32)
    selfm = src0 == dst0
    np.add.at(mult, dst0[selfm], 1.0)

    xs = (x * dinv[:, None]).astype(ml_dtypes.bfloat16)  # gather table rows

    cross = ~selfm
    src = src0[cross]
    dst = dst0[cross]

    core = dst // per
    fat = (dst % per) // gf
    wsrc = src // win
    lid = (dst % per) % gf

    # counts per (core, fat, wsrc) -> shared column counts (max over cores)
    counts = np.zeros((nc_, ngf, nwin), dtype=np.int64)
    np.add.at(counts, (core, fat, wsrc), 1)
    cols_fw = np.ceil(counts.max(axis=0) / g).astype(np.int64)  # [ngf, nwin]
    assert (counts > 0).all(), "empty (core,fat,wsrc) bucket"

    # per-core cumulative lid-window counts inside each bucket (for w-sets)
    cnt_lw = np.zeros((nc_, ngf, nwin, nlw), dtype=np.int64)
    np.add.at(cnt_lw, (core, fat, wsrc, lid // g), 1)
    cum_lw = np.zeros((nc_, ngf, nwin, nlw + 1), dtype=np.int64)
    cum_lw[..., 1:] = np.cumsum(cnt_lw, axis=-1)

    # slot layout: for chunk: for wsrc: for fat in chunk: cols_fw[fat,wsrc]*g
    chunks = [
        list(range(k0, min(k0 + cfg.chunk_f, ngf)))
        for k0 in range(0, ngf, cfg.chunk_f)
    ]
    region_off = {}      # (fat, wsrc) -> slot offset
    call_meta = []       # per (chunk, wsrc): (sl0, nsl, [(fat, col0, ncols)])
    off = 0
    for K in chunks:
        for w in range(nwin):
            sl0 = off
            items = []
            for f in K:
                ncol = int(cols_fw[f, w])
                region_off[(f, w)] = off
                items.append((f, (off - sl0) // g, ncol))
                off += ncol * g
            call_meta.append((sl0, off - sl0, items))
    tot_slots = off
    tot_cols = tot_slots // g

    # subcolumn structure: per (fat, wsrc, col): sorted list of lid-windows
    # (union over cores of windows overlapping the column's slot range)
    sub_ws = {}
    real_rows = [min(gf, per - f * gf) for f in range(ngf)]
    nsub_tot = 0
    for K in chunks:
        for w in range(nwin):
            for f in K:
                ncol = int(cols_fw[f, w])
                for j in range(ncol):
                    lo, hi = j * g, j * g + g
                    ws = set()
                    for c in range(nc_):
                        cc = cum_lw[c, f, w]
                        for lw in range(nlw):
                            if cc[lw] < hi and cc[lw + 1] > lo:
                                ws.add(lw)
                    wl = sorted(ws)
                    assert wl, (f, w, j)
                    sub_ws[(f, w, j)] = wl
                    nsub_tot += len(wl)
    # every real region must receive at least one matmul (PSUM init)
    covered = set()
    for (f, w, j), wl in sub_ws.items():
        for lw in wl:
            covered.add((f, lw))
    for f in range(ngf):
        for lw in range(math.ceil(real_rows[f] / g)):
            assert (f, lw) in covered, (f, lw)

    # ---- per-core idx / lidloc arrays ----
    order = np.lexsort((lid, wsrc, fat, core))
    src_s = src[order]
    core_s, fat_s, w_s, lid_s = core[order], fat[order], wsrc[order], lid[order]

    idx_all = np.zeros((nc_, tot_slots), dtype=np.int16)
    lid_all = np.full((nc_, tot_slots), 4 * g, dtype=np.int64)  # pad sentinel
    for c in range(nc_):
        m = core_s == c
        sc_, fc, wc, lc = src_s[m], fat_s[m], w_s[m], lid_s[m]
        key = fc * nwin + wc
        change = np.r_[True, key[1:] != key[:-1]]
        run_id = np.cumsum(change) - 1
        run_start = np.flatnonzero(change)
        rank = np.arange(len(key)) - run_start[run_id]
        base = np.array(
            [region_off[(ff, ww)] for ff, ww in zip(fc[change], wc[change])]
        )
        slot = base[run_id] + rank
        idx_all[c, slot] = (sc_ - wc * win).astype(np.int16)
        lid_all[c, slot] = lc

    # idx wrapped [128, tot_slots//16] (16-partition wrap, replicated x8)
    idx_wrap = np.zeros((nc_, 128, tot_slots // 16), dtype=np.int16)
    for c in range(nc_):
        wrapped = idx_all[c].reshape(-1, 16).T
        idx_wrap[c] = np.tile(wrapped, (8, 1))

    # lid2 stream: per subcol (ordered like the program consumes them):
    # 128 lidloc values pair-duplicated -> [128, 2*nsub_tot]
    lid2 = np.zeros((nc_, 128, 2 * nsub_tot), dtype=ml_dtypes.bfloat16)
    mm_meta = []  # per call: [(colpos_in_call, lw, fat, subidx)]
    sidx = 0
    ci = 0
    for K in chunks:
        for w in range(nwin):
            sl0, nsl, items = call_meta[ci]
            ci += 1
            mlist = []
            for f, col0, ncol in items:
                for j in range(ncol):
                    slot0 = region_off[(f, w)] + j * g
                    col_lids = lid_all[:, slot0 : slot0 + g]  # [nc, 128]
                    for lw in sub_ws[(f, w, j)]:
                        ll = col_lids - lw * g
                        ll = np.where((ll >= 0) & (ll < g), ll, 255)
                        v = ll.astype(ml_dtypes.bfloat16)  # [nc, 128]
                        lid2[:, :, 2 * sidx] = v
                        lid2[:, :, 2 * sidx + 1] = v
                        mlist.append((col0 + j, lw, f, sidx))
                        sidx += 1
            mm_meta.append(mlist)
    assert sidx == nsub_tot

    # ---- small constants ----
    sub = cfg.sub
    iota_rep = np.tile(
        np.tile(np.arange(g, dtype=np.float32), sub), (128, 1)
    ).astype(ml_dtypes.bfloat16)  # [128, sub*g]
    W1bf = W1.astype(ml_dtypes.bfloat16)  # [nin, hid]
    W2p = np.zeros((cfg.hid, cfg.hid), dtype=np.float32)
    W2p[:, : cfg.outf] = W2
    W2bf = W2p.astype(ml_dtypes.bfloat16)
    b1col = b1.reshape(cfg.hid, 1).astype(np.float32)
    b2bc = np.zeros((128, cfg.hid), dtype=np.float32)
    b2bc[:, : cfg.outf] = b2[None, :]
    hasb1 = bool(np.any(b1))
    hasb2 = bool(np.any(b2))
    ident64 = np.eye(cfg.hid, dtype=np.float32).astype(ml_dtypes.bfloat16)

    perp, nreg = cfg.perp, cfg.nreg
    in_maps = []
    for c in range(nc_):
        lo, hi = c * per, (c + 1) * per
        dslice = np.concatenate([dinv[lo:hi], np.ones(perp - per, np.float32)])
        mslice = np.concatenate([mult[lo:hi], np.zeros(perp - per, np.float32)])
        # feature-major own x slice, pre-scaled by dinv*mult (self term)
        xso = np.zeros((cfg.nin, perp), dtype=np.float32)
        xso[:, :per] = (x[lo:hi] * (dinv[lo:hi] * mult[lo:hi])[:, None]).T
        dbase = dslice if hasb1 else dslice * dslice
        dinv_bc = np.tile(dbase, (cfg.hid, 1)).astype(ml_dtypes.bfloat16)
        # node-major [128, nreg] scale tables for the layer-2 epilogue
        dcol = dslice.reshape(nreg, g).T.copy()
        scol = (dslice * mslice).reshape(nreg, g).T.copy()
        in_maps.append(
            {
                "xs": np.asarray(xs),
                "xsT_own": np.asarray(xso.astype(ml_dtypes.bfloat16)),
                "W1bf": np.asarray(W1bf),
                "W2bf": np.asarray(W2bf),
                "b1col": b1col,
                "b2bc": b2bc,
                "iota": np.asarray(iota_rep),
                "ident64": np.asarray(ident64),
                "idx": idx_wrap[c],
                "lid2": np.asarray(lid2[c]),
                "dinv_bc": np.asarray(dinv_bc),
                "dinv_col": dcol,
                "self_col": scol,
            }
        )

    sched = dict(
        call_meta=call_meta,
        chunks=chunks,
        mm_meta=mm_meta,
        tot_slots=tot_slots,
        tot_cols=tot_cols,
        nsub_tot=nsub_tot,
        hasb1=hasb1,
        hasb2=hasb2,
    )
    return in_maps, sched


# ---------------------------------------------------------------- device side


def _build(cfg: Cfg, sched) -> bacc.Bacc:
    n, hid, g, gf, nwin, win = cfg.n, cfg.hid, cfg.g, cfg.gf, cfg.nwin, cfg.win
    ngf, per, perp, nreg, nlw = cfg.ngf, cfg.per, cfg.perp, cfg.nreg, cfg.nlw
    nin, of_ = cfg.nin, cfg.outf
    call_meta, chunks, mm_meta = sched["call_meta"], sched["chunks"], sched["mm_meta"]
    hasb1, hasb2 = sched["hasb1"], sched["hasb2"]
    tot_slots, nsub_tot = sched["tot_slots"], sched["nsub_tot"]
    max_call_cols = max((m[1] // g for m in call_meta), default=1)

    nc = bacc.Bacc("TRN2", target_bir_lowering=False, debug=False,
                   num_devices=cfg.ncores)

    xs_d = nc.dram_tensor("xs", [n, nin], BF16, kind="ExternalInput").ap()
    xso_d = nc.dram_tensor("xsT_own", [nin, perp], BF16, kind="ExternalInput").ap()
    W1bf = nc.dram_tensor("W1bf", [nin, hid], BF16, kind="ExternalInput").ap()
    W2bf = nc.dram_tensor("W2bf", [hid, hid], BF16, kind="ExternalInput").ap()
    b1col = nc.dram_tensor("b1col", [hid, 1], F32, kind="ExternalInput").ap()
    b2bc = nc.dram_tensor("b2bc", [128, hid], F32, kind="ExternalInput").ap()
    iota_d = nc.dram_tensor("iota", [128, cfg.sub * g], BF16, kind="ExternalInput").ap()
    ident_d = nc.dram_tensor("ident64", [hid, hid], BF16, kind="ExternalInput").ap()
    idx_d = nc.dram_tensor("idx", [128, tot_slots // 16], I16, kind="ExternalInput").ap()
    lid2_d = nc.dram_tensor("lid2", [128, 2 * nsub_tot], BF16, kind="ExternalInput").ap()
    dinvbc_d = nc.dram_tensor("dinv_bc", [hid, perp], BF16, kind="ExternalInput").ap()
    dinvcol_d = nc.dram_tensor("dinv_col", [128, nreg], F32, kind="ExternalInput").ap()
    selfcol_d = nc.dram_tensor("self_col", [128, nreg], F32, kind="ExternalInput").ap()

    out_d = nc.dram_tensor("out", [per, of_], F32, kind="ExternalOutput").ap()

    h2c_b = nc.dram_tensor("h2c_b", [per, of_], BF16).ap()   # AllGather input
    T3c = nc.dram_tensor("T3c", [n, of_], BF16, addr_space="Shared").ap()
    T3 = nc.dram_tensor("T3", [n, nin], BF16).ap()           # 256B-row table

    from concourse import library_config

    with tile.TileContext(nc) as tc, ExitStack() as ctx:
        nc.gpsimd.load_library(library_config.mlp)

        consts = ctx.enter_context(tc.tile_pool(name="consts", bufs=1))
        sb = ctx.enter_context(tc.tile_pool(name="sb", bufs=3))
        mtp = ctx.enter_context(tc.tile_pool(name="mtp", bufs=5))
        idxp = ctx.enter_context(tc.tile_pool(name="idxp", bufs=6))
        subp = ctx.enter_context(tc.tile_pool(name="subp", bufs=2))
        eptmp = ctx.enter_context(tc.tile_pool(name="eptmp", bufs=2))
        fep = ctx.enter_context(tc.tile_pool(name="fep", bufs=1))
        psum_acc = ctx.enter_context(tc.tile_pool(name="psuma", bufs=5, space="PSUM"))
        psum_mm = ctx.enter_context(tc.tile_pool(name="psummm", bufs=2, space="PSUM"))

        # resident constants
        w1_t = consts.tile([nin, hid], BF16)
        nc.sync.dma_start(w1_t[:], W1bf[:, :])
        w2_t = consts.tile([hid, hid], BF16)
        nc.sync.dma_start(w2_t[:], W2bf[:, :])
        b1_t = consts.tile([hid, 1], F32)
        nc.sync.dma_start(b1_t[:], b1col[:, :])
        b2_t = consts.tile([128, hid], F32)
        nc.sync.dma_start(b2_t[:], b2bc[:, :])
        iota_t = consts.tile([128, cfg.sub * g], BF16)
        nc.sync.dma_start(iota_t[:], iota_d[:, :])
        ident_t = consts.tile([hid, hid], BF16)
        nc.sync.dma_start(ident_t[:], ident_d[:, :])
        dinvbc_t = consts.tile([hid, perp], BF16)
        nc.sync.dma_start(dinvbc_t[:], dinvbc_d[:, :])
        dinvcol_t = consts.tile([128, nreg], F32)
        nc.sync.dma_start(dinvcol_t[:], dinvcol_d[:, :])
        selfcol_t = consts.tile([128, nreg], F32)
        nc.sync.dma_start(selfcol_t[:], selfcol_d[:, :])
        lid2_t = consts.tile([128, 2 * nsub_tot], BF16)
        nc.sync.dma_start(lid2_t[:], lid2_d[:, :])

        t1own = consts.tile([hid, perp], BF16)      # self rows mult*dinv*h1
        h2stage = consts.tile([128, nreg, hid], BF16)   # local h2 rows (nm)

        # ---------------- t1own = (mult*dinv*x own)^T @ W1, feature-major
        for f0 in range(0, perp, gf):
            xso_t = sb.tile([nin, gf], BF16, tag="xso")
            nc.sync.dma_start(xso_t[:], xso_d[:, f0 : f0 + gf])
            pt = psum_mm.tile([hid, gf], F32, tag="mm", name="t1own_p")
            nc.tensor.matmul(
                out=pt[:], lhsT=w1_t[:], rhs=xso_t[:],
                start=True, stop=True,
            )
            nc.vector.tensor_copy(t1own[:, f0 : f0 + gf], pt[:])

        # ---------------- batched layer-2 epilogue + log_softmax (per
        # region chunk, interleaved with layer-2 aggregation)
        def final_ep(a2c, r0, rn):
            a2 = a2c[:, :rn, :]
            nc.vector.tensor_tensor(
                out=a2,
                in0=a2,
                in1=dinvcol_t[:, r0 : r0 + rn]
                .unsqueeze(2)
                .broadcast_to([128, rn, hid]),
                op=ALU.mult,
            )
            o2 = fep.tile([128, rn, hid], F32, tag="fe2", name="o2")
            nc.vector.tensor_tensor(
                out=o2[:],
                in0=h2stage[:, r0 : r0 + rn, :],
                in1=selfcol_t[:, r0 : r0 + rn]
                .unsqueeze(2)
                .broadcast_to([128, rn, hid]),
                op=ALU.mult,
            )
            nc.vector.tensor_add(a2, a2, o2[:])
            if hasb2:
                nc.vector.tensor_tensor(
                    out=a2,
                    in0=a2,
                    in1=b2_t[:].unsqueeze(1).broadcast_to([128, rn, hid]),
                    op=ALU.add,
                )
            nmax = fep.tile([128, rn, 1], F32, tag="fm")
            nc.vector.tensor_reduce(
                nmax[:], a2[:, :, :of_], mybir.AxisListType.X,
                ALU.max, negate=True,
            )
            sh = fep.tile([128, rn, of_], F32, tag="fe3", name="sh")
            nc.vector.tensor_tensor(
                out=sh[:],
                in0=a2[:, :, :of_],
                in1=nmax[:].broadcast_to([128, rn, of_]),
                op=ALU.add,
            )
            ex = o2[:, :rn, :of_]
            nc.scalar.activation(ex, sh[:], AF.Exp)
            sm = fep.tile([128, rn, 1], F32, tag="fs")
            nc.vector.tensor_reduce(
                sm[:], ex, mybir.AxisListType.X, ALU.add
            )
            ls = fep.tile([128, rn, 1], F32, tag="fl")
            nc.scalar.activation(
                ls[:].rearrange("p q h -> p (q h)"),
                sm[:].rearrange("p q h -> p (q h)"),
                AF.Ln,
            )
            fin = sh
            nc.vector.tensor_tensor(
                out=fin[:],
                in0=sh[:],
                in1=ls[:].broadcast_to([128, rn, of_]),
                op=ALU.subtract,
            )
            # store: full 128-regions fused, partial region separately
            nf = max(0, min(per // g - r0, rn))
            if nf:
                nc.sync.dma_start(
                    out_d[r0 * g : (r0 + nf) * g, :].rearrange(
                        "(q p) f -> p q f", p=128
                    ),
                    fin[:, :nf, :],
                )
            pi = per // g  # partial region index
            if r0 <= pi < r0 + rn and per % g:
                nc.sync.dma_start(
                    out_d[pi * g : per, :], fin[: per % g, pi - r0, :]
                )

        # ---------------- aggregation layers
        def agg_layer(layer: int, table_ap):
            ci = 0
            for K in chunks:
                # one PSUM bank per fat group (layer 1) / half bank (layer 2)
                if layer == 1:
                    banks = {
                        f: psum_acc.tile([128, gf], F32, tag="acc", name=f"a1_{f}")
                        for f in K
                    }

                    def acc_ap(f, lw):
                        return banks[f][:, lw * g : (lw + 1) * g]
                else:
                    bt = {}
                    for i in range(0, len(K), 2):
                        t = psum_acc.tile([128, 512], F32, tag="acc",
                                          name=f"a2_{K[i]}")
                        for j, f in enumerate(K[i : i + 2]):
                            bt[f] = (t, j)
                    banks = bt

                    def acc_ap(f, lw):
                        t, j = banks[f]
                        return t[:, (j * nlw + lw) * hid : (j * nlw + lw + 1) * hid]

                # per-bank first/last matmul bookkeeping
                def bank_key(f):
                    return id(banks[f]) if layer == 1 else id(banks[f][0])

                tot_bank = {}
                for w in range(nwin):
                    for cp, lw, f, si in mm_meta[ci + w]:
                        tot_bank[bank_key(f)] = tot_bank.get(bank_key(f), 0) + 1
                emitted = dict.fromkeys(tot_bank, 0)

                for w in range(nwin):
                    sl0, nsl, items = call_meta[ci]
                    mlist = mm_meta[ci]
                    ci += 1
                    cols = nsl // g
                    it = idxp.tile([128, max_call_cols * 8], I16, tag="idx")
                    nc.sync.dma_start(
                        it[:, : nsl // 16], idx_d[:, sl0 // 16 : (sl0 + nsl) // 16]
                    )
                    mt = mtp.tile([128, max_call_cols, nin], BF16, tag="m")
                    nc.gpsimd.dma_gather(
                        mt[:, :cols, :],
                        table_ap[w * win : (w + 1) * win, :],
                        it[:, : nsl // 16],
                        nsl,
                        nsl,
                        nin,
                        single_packet=False,
                    )
                    # S slabs over this call's subcol range
                    si0 = mlist[0][3]
                    nsub = len(mlist)
                    for s0 in range(0, nsub, cfg.sub):
                        sc = min(cfg.sub, nsub - s0)
                        st_ = subp.tile([128, cfg.sub * g], BF16, tag="sel")
                        l2 = lid2_t[
                            :, 2 * (si0 + s0) : 2 * (si0 + s0 + sc)
                        ].rearrange("p (c t) -> p c t", t=2)
                        nc.vector.tensor_tensor(
                            out=st_[:, : sc * g].rearrange(
                                "p (c r t) -> p c r t", r=g // 2, t=2
                            ),
                            in0=iota_t[:, : sc * g].rearrange(
                                "p (c r t) -> p c r t", r=g // 2, t=2
                            ),
                            in1=l2.unsqueeze(2).broadcast_to([128, sc, g // 2, 2]),
                            op=ALU.is_equal,
                        )
                        for k in range(sc):
                            cp, lw, f, si = mlist[s0 + k]
                            bk = bank_key(f)
                            first = emitted[bk] == 0
                            emitted[bk] += 1
                            last = emitted[bk] == tot_bank[bk]
                            if layer == 1:
                                nc.tensor.matmul(
                                    out=acc_ap(f, lw),
                                    lhsT=mt[:, cp, :],
                                    rhs=st_[:, k * g : (k + 1) * g],
                                    start=first,
                                    stop=last,
                                )
                            else:
                                nc.tensor.matmul(
                                    out=acc_ap(f, lw),
                                    lhsT=st_[:, k * g : (k + 1) * g],
                                    rhs=mt[:, cp, :hid],
                                    start=first,
                                    stop=last,
                                )

                # epilogues for chunk K
                if layer == 2:
                    a2c = fep.tile(
                        [128, cfg.chunk_f * nlw, hid], F32, tag="a2c", name="a2c"
                    )
                for f in K:
                    c0 = f * gf
                    if layer == 1:
                        accS = eptmp.tile([128, gf], BF16, tag="ep0")
                        nc.vector.tensor_copy(accS[:], banks[f][:])
                        hp = psum_mm.tile([hid, gf], F32, tag="mm", name="h1pre")
                        nc.tensor.matmul(
                            out=hp[:], lhsT=w1_t[:], rhs=accS[:],
                            start=True, stop=False,
                        )
                        nc.tensor.matmul(
                            out=hp[:], lhsT=ident_t[:],
                            rhs=t1own[:, c0 : c0 + gf],
                            start=False, stop=True,
                        )
                        if hasb1:
                            t2_ = eptmp.tile([hid, gf], F32, tag="ep2")
                            nc.vector.tensor_mul(
                                t2_[:], hp[:], dinvbc_t[:, c0 : c0 + gf]
                            )
                            t3_ = eptmp.tile([hid, gf], F32, tag="ep3")
                            nc.scalar.activation(
                                t3_[:], t2_[:], AF.Relu, bias=b1_t[:, :1]
                            )
                            t2p = eptmp.tile([hid, gf], BF16, tag="ep4")
                            nc.vector.tensor_mul(
                                t2p[:], t3_[:], dinvbc_t[:, c0 : c0 + gf]
                            )
                        else:
                            # b1 == 0: relu(dinv*u)*dinv == relu(u)*dinv^2
                            # (dinvbc_t holds dinv^2 in this mode)
                            t3_ = eptmp.tile([hid, gf], BF16, tag="ep3")
                            nc.scalar.activation(t3_[:], hp[:], AF.Relu)
                            t2p = eptmp.tile([hid, gf], BF16, tag="ep4")
                            nc.vector.tensor_mul(
                                t2p[:], t3_[:], dinvbc_t[:, c0 : c0 + gf]
                            )
                        # W2 matmuls -> node-major h2 staging
                        pw = psum_mm.tile([128, nlw * hid], F32, tag="mm", name="pw")
                        for q in range(nlw):
                            nc.tensor.matmul(
                                out=pw[:, q * hid : (q + 1) * hid],
                                lhsT=t2p[:, q * g : (q + 1) * g],
                                rhs=w2_t[:],
                                start=True,
                                stop=True,
                            )
                        nc.vector.tensor_copy(
                            h2stage[:, f * nlw : (f + 1) * nlw, :].rearrange(
                                "p q h -> p (q h)"
                            ),
                            pw[:],
                        )
                    else:
                        t, j = banks[f]
                        jj = f - K[0]
                        nc.vector.tensor_copy(
                            a2c[:, jj * nlw : (jj + 1) * nlw, :].rearrange(
                                "p q h -> p (q h)"
                            ),
                            t[:, j * nlw * hid : (j + 1) * nlw * hid],
                        )
                if layer == 2:
                    final_ep(a2c, K[0] * nlw, len(K) * nlw)

        _phases = int(os.environ.get("GCN_PHASES", "4"))
        if _phases >= 2:
            agg_layer(1, xs_d)

        # ---------------- ship compact h2, AllGather, expand table
        if _phases >= 3:
            nfull = per // g
            rem = per - nfull * g
            nc.sync.dma_start(
                h2c_b[: nfull * g, :].rearrange("(q p) h -> p q h", p=128),
                h2stage[:, :nfull, :of_],
            )
            if rem:
                nc.sync.dma_start(
                    h2c_b[nfull * g : per, :], h2stage[:rem, nfull, :of_]
                )

        if _phases >= 3 and not os.environ.get("GCN_NO_COLL"):
            nc.gpsimd.collective_compute(
                "AllGather",
                ALU.bypass,
                replica_groups=[list(range(cfg.ncores))],
                ins=[h2c_b.opt()],
                outs=[T3c.opt()],
            )
            # expand compact rows into the 256B-row gather table, one src
            # window at a time so layer 2 can start on window 0 early.
            for r0 in range(0, n, win):
                nc.sync.dma_start(T3[r0 : r0 + win, :of_], T3c[r0 : r0 + win, :])

        if _phases >= 4:
            agg_layer(2, T3)

    nc.compile()
    return nc


# ---------------------------------------------------------------- entry


def kernel(x, edge_index, W1, b1, W2, b2, cfg: Cfg | None = None, _run=None):
    cfg = cfg or Cfg()
    in_maps, sched = _preprocess(
        np.asarray(x), np.asarray(edge_index), np.asarray(W1), np.asarray(b1),
        np.asarray(W2), np.asarray(b2), cfg
    )
    nc = _build(cfg, sched)
    if _run is not None:  # test hook (e.g. simulator)
        results = _run(nc, in_maps)
    else:
        results = run_bass_kernel_spmd(
            nc, in_maps, core_ids=list(range(cfg.ncores))
        ).results
    out = np.concatenate([results[c]["out"] for c in range(cfg.ncores)], axis=0)
    return out.astype(np.float32)
